# revision 5
# baseline (speedup 1.0000x reference)
"""Distributed permutohedral-lattice splat (scatter-add) for 8 Trainium2 cores.

Strategy (data-parallel over points, per the sharding hint):
  - Each of the 8 NeuronCores gets 1/8 of the points (padded + masked).
  - On-core: the permutohedral slot/weight math runs in f32 on the vector
    engine (op-for-op mirror of the reference, incl. the uint32 hash done in
    exact-f32 limb arithmetic mod 2^20), laid out free-major
    [128 lanes x 128 points] and PE-transposed to point-major.
  - The scatter-add runs as 4 independent serial gather-combine-scatter
    chains (chain k = simplex vertex k) into 4 per-core partial tables.
    Within a 128-row chunk, duplicate slots are merged with a selection-
    matrix matmul (rows with equal slots all receive the full sum, so
    colliding DMA writes are identical); across chunks a chain is
    serialized by the table RAW/WAW dependency; across chains the tables
    are disjoint, so no ordering is needed.
  - The 4 partial tables are summed on-device; the 8 per-core tables are
    summed on the host (the all-reduce step of the hint, folded into the
    unshard step).
"""

import os
os.environ["NEURON_SCRATCHPAD_PAGE_SIZE"] = "2048"
import numpy as np
from contextlib import ExitStack

import concourse.bass as bass
import concourse.tile as tile
from concourse import bacc, mybir
from concourse._compat import with_exitstack

F32 = mybir.dt.float32
I32 = mybir.dt.int32
AOT = mybir.AluOpType

D = 3
DP1 = 4
CAP = 1 << 20
MAGIC = 12582912.0            # 1.5 * 2^23 : round-to-nearest-even trick for |x| < 2^22
HMUL = 2531011
C20 = HMUL % CAP
B20 = (HMUL * HMUL) % (1 << 32) % CAP
A20 = ((HMUL * HMUL) % (1 << 32)) * HMUL % (1 << 32) % CAP
MULTS = [A20, B20, C20]       # slot = (k0*A20 + k1*B20 + k2*C20) mod 2^20
SCALES = [float(np.float32(np.sqrt(2.0 / 3.0) * DP1 / np.sqrt((i + 1.0) * (i + 2.0)))) for i in range(D)]


def build(nc, NP, n_merge_free=1024, unroll=8, gather_bufs=2, loop_mode="for_i_unrolled", z_reps=1, h_reps=1, s_reps=1, m_reps=1, cce=False, tabs_external=False, do_zero=True, do_merge=True):
    """NP must be a multiple of 16384. Returns nothing; program built into nc."""
    assert NP % 16384 == 0
    if not do_merge or not do_zero:
        # un-merged partial tables must be returned to the host for the final
        # sum; un-zeroed tables rely on run_bass_kernel_spmd's donated
        # zero-initialized ExternalOutput buffers.
        assert tabs_external
    NT = NP // 16384              # hash tiles
    NCH = NP // 128               # point-chunks (columns in slotT/wT)

    pos = nc.dram_tensor("positions", [NP * 3], F32, kind="ExternalInput").ap()
    vals = nc.dram_tensor("values", [NP, 64], F32, kind="ExternalInput").ap()
    msk = nc.dram_tensor("mask", [NP], F32, kind="ExternalInput").ap()
    ident = nc.dram_tensor("ident", [128, 128], F32, kind="ExternalInput").ap()
    ltm = nc.dram_tensor("ltm", [128, 128], F32, kind="ExternalInput").ap()
    # cce mode: +128 trash rows per table — within-chunk duplicate rows are
    # scattered to row CAP+partition instead of being OOB-dropped (descriptor
    # drops upset DMA completion accounting on HW).
    TR = 128 if cce else 0
    out = nc.dram_tensor("out", [CAP + TR, 65], F32, kind="ExternalOutput").ap()
    tab_kind = "ExternalOutput" if tabs_external else "Internal"
    tabs = [out] + [
        nc.dram_tensor(f"tab{k}", [CAP + TR, 65], F32, kind=tab_kind).ap()
        for k in range(1, DP1)
    ]
    if cce:
        capp = nc.dram_tensor("capp", [128, 1], F32, kind="ExternalInput").ap()

    with tile.TileContext(nc) as tc:
        with ExitStack() as ctx:
            resident = ctx.enter_context(tc.tile_pool(name="resident", bufs=1))
            identity = resident.tile([128, 128], F32, tag="ident", name="ident" + '_1')
            nc.sync.dma_start(identity[:], ident[:])
            ltmask = resident.tile([128, 128], F32, tag="ltm", name="ltm")
            nc.sync.dma_start(ltmask[:], ltm[:])
            if cce:
                cappt = resident.tile([128, 1], F32, tag="capp", name="cappt")
                nc.sync.dma_start(cappt[:], capp[:])

            slotT_f = [resident.tile([128, NCH], F32, tag=f"sf{k}", name=f"sf{k}" + '_2') for k in range(DP1)]
            slotT_i = [resident.tile([128, NCH], I32, tag=f"si{k}", name=f"si{k}" + '_3') for k in range(DP1)] if not cce else None
            wT = [resident.tile([128, NCH], F32, tag=f"w{k}", name=f"w{k}" + '_4') for k in range(DP1)]

            # ---- memset all tables (incl. out: no reliance on harness zero-init) ----
            zpool = ctx.enter_context(tc.tile_pool(name="zpool", bufs=1))
            ztile = zpool.tile([128, 4096], F32, name="ztile")
            nc.vector.memset(ztile[:], 0.0)
            total = CAP * 65                      # f32 elements per table
            zchunk = 128 * 4096
            nzfull = total // zchunk              # 130 full chunks
            zrem = total - nzfull * zchunk        # remainder elements
            for _zr in range(z_reps if do_zero else 0):
             for k in range(0, DP1):
                flat = tabs[k][0:CAP, :].rearrange("v d -> (v d)")
                for i in range(nzfull):
                    nc.sync.dma_start(
                        flat[i * zchunk : (i + 1) * zchunk].rearrange("(p f) -> p f", p=128),
                        ztile[:],
                    )
                if zrem:
                    assert zrem % 128 == 0
                    nc.sync.dma_start(
                        flat[nzfull * zchunk :].rearrange("(p f) -> p f", p=128),
                        ztile[:, : zrem // 128],
                    )

            # ================= Phase H =================
            hctx = ExitStack()
            hp = hctx.enter_context(tc.tile_pool(name="hash", bufs=2))
            hpsum = hctx.enter_context(tc.tile_pool(name="hpsum", bufs=4, space="PSUM"))

            def TT(tag):
                return hp.tile([128, 128], F32, tag=tag, name=tag)

            def ts(out_, in_, s0, op0, s1=None, op1=None):
                if s1 is None:
                    nc.vector.tensor_scalar(out_, in_, s0, None, op0)
                else:
                    nc.vector.tensor_scalar(out_, in_, s0, s1, op0, op1)

            def tt(out_, a, b, op):
                nc.vector.tensor_tensor(out=out_, in0=a, in1=b, op=op)

            def stt(out_, in0, s, op0, in1, op1):
                nc.vector.scalar_tensor_tensor(out=out_, in0=in0, scalar=s, in1=in1, op0=op0, op1=op1)

            def f_round(dst, src):      # dst = rne(src), |src| < 2^22
                ts(dst, src, MAGIC, AOT.add)
                ts(dst, dst[:], MAGIC, AOT.subtract)

            for _hr in range(h_reps):
             for h in range(NT):
                ptile = hp.tile([128, 384], F32, tag="pos", name="pos" + '_5')
                nc.sync.dma_start(ptile[:], pos[h * 49152 : (h + 1) * 49152].rearrange("(p f) -> p f", p=128))
                p3 = ptile[:].rearrange("p (t c) -> p t c", c=3)

                c = [TT(f"c{i}") for i in range(3)]
                for i in range(3):
                    ts(c[i][:], p3[:, :, i], SCALES[i], AOT.mult)

                e = [TT(f"e{i}") for i in range(4)]
                # s2=c2; s1=c1+c2; s0=c0+s1; e=[s0, s1-c0, c2-2c1, -3c2]
                tt(e[1][:], c[1][:], c[2][:], AOT.add)            # e1 <- s1
                tt(e[0][:], c[0][:], e[1][:], AOT.add)            # e0 <- s0
                tt(e[1][:], e[1][:], c[0][:], AOT.subtract)       # e1 = s1 - c0
                stt(e[2][:], c[1][:], -2.0, AOT.mult, c[2][:], AOT.add)   # e2 = c2 - 2c1
                ts(e[3][:], c[2][:], -3.0, AOT.mult)              # e3 = -3c2

                rem = [TT(f"rem{i}") for i in range(4)]
                dif = [TT(f"dif{i}") for i in range(4)]
                t1 = TT("t1"); t2 = TT("t2"); t3 = TT("t3"); t4 = TT("t4")
                for i in range(4):
                    ts(t1[:], e[i][:], 0.25, AOT.mult)            # v
                    f_round(t2[:], t1[:])                          # tr
                    tt(t3[:], t2[:], t1[:], AOT.is_gt)            # tr > v
                    tt(t3[:], t2[:], t3[:], AOT.subtract)         # fl = tr - (tr>v)
                    tt(t4[:], t2[:], t1[:], AOT.is_lt)            # tr < v
                    tt(t4[:], t2[:], t4[:], AOT.add)              # ce = tr + (tr<v)
                    ts(t3[:], t3[:], 4.0, AOT.mult)               # down
                    ts(t4[:], t4[:], 4.0, AOT.mult)               # up
                    tt(t2[:], t4[:], e[i][:], AOT.subtract)       # up - e
                    tt(t1[:], e[i][:], t3[:], AOT.subtract)       # e - down
                    tt(t2[:], t2[:], t1[:], AOT.is_lt)            # pick up?
                    stt(rem[i][:], t2[:], 4.0, AOT.mult, t3[:], AOT.add)  # rem = down + 4*pick
                    tt(dif[i][:], e[i][:], rem[i][:], AOT.subtract)

                # ranks
                lt = {}
                for i in range(4):
                    for j in range(i + 1, 4):
                        lt[(i, j)] = TT(f"lt{i}{j}")
                        tt(lt[(i, j)][:], dif[i][:], dif[j][:], AOT.is_lt)
                r = [TT(f"r{i}") for i in range(4)]
                tt(r[0][:], lt[(0, 1)][:], lt[(0, 2)][:], AOT.add)
                tt(r[0][:], r[0][:], lt[(0, 3)][:], AOT.add)
                tt(r[1][:], lt[(1, 2)][:], lt[(1, 3)][:], AOT.add)
                ts(t1[:], lt[(0, 1)][:], -1.0, AOT.mult, 1.0, AOT.add)
                tt(r[1][:], r[1][:], t1[:], AOT.add)
                ts(t1[:], lt[(0, 2)][:], -1.0, AOT.mult, 2.0, AOT.add)
                tt(t1[:], t1[:], lt[(1, 2)][:], AOT.subtract)
                tt(r[2][:], t1[:], lt[(2, 3)][:], AOT.add)
                tt(t1[:], lt[(0, 3)][:], lt[(1, 3)][:], AOT.add)
                tt(t1[:], t1[:], lt[(2, 3)][:], AOT.add)
                ts(r[3][:], t1[:], -1.0, AOT.mult, 3.0, AOT.add)

                # sum_rem/4 ; shifts
                tt(t1[:], rem[0][:], rem[1][:], AOT.add)
                tt(t1[:], t1[:], rem[2][:], AOT.add)
                tt(t1[:], t1[:], rem[3][:], AOT.add)
                ts(t1[:], t1[:], 0.25, AOT.mult)                  # sum_rem
                for i in range(4):
                    tt(r[i][:], r[i][:], t1[:], AOT.add)
                for i in range(4):
                    ts(t2[:], r[i][:], 0.0, AOT.is_lt)            # rank < 0
                    ts(t3[:], r[i][:], 3.0, AOT.is_gt)            # rank > 3
                    stt(rem[i][:], t2[:], 4.0, AOT.mult, rem[i][:], AOT.add)
                    stt(rem[i][:], t3[:], -4.0, AOT.mult, rem[i][:], AOT.add)
                    stt(r[i][:], t2[:], 4.0, AOT.mult, r[i][:], AOT.add)
                    stt(r[i][:], t3[:], -4.0, AOT.mult, r[i][:], AOT.add)

                delta = [TT(f"dl{i}") for i in range(4)]
                for i in range(4):
                    tt(delta[i][:], e[i][:], rem[i][:], AOT.subtract)
                    ts(delta[i][:], delta[i][:], 0.25, AOT.mult)

                # weights: sel(r) = sum_i delta_i * (rank_i == r)
                sels = []
                for rv in range(4):
                    acc = TT(f"sel{rv}")
                    for i in range(4):
                        ts(t1[:], r[i][:], float(rv), AOT.is_equal)
                        tt(t1[:], t1[:], delta[i][:], AOT.mult)
                        if i == 0:
                            nc.vector.tensor_copy(acc[:], t1[:])
                        else:
                            tt(acc[:], acc[:], t1[:], AOT.add)
                    sels.append(acc)
                mtile = hp.tile([128, 128], F32, tag="msk", name="msk" + '_6')
                nc.sync.dma_start(mtile[:], msk[h * 16384 : (h + 1) * 16384].rearrange("(p f) -> p f", p=128))
                w = [TT(f"wv{k}") for k in range(4)]
                ts(t1[:], sels[0][:], -1.0, AOT.mult, 1.0, AOT.add)
                tt(w[0][:], sels[3][:], t1[:], AOT.add)
                tt(w[1][:], sels[2][:], sels[3][:], AOT.subtract)
                tt(w[2][:], sels[1][:], sels[2][:], AOT.subtract)
                tt(w[3][:], sels[0][:], sels[1][:], AOT.subtract)
                for k in range(4):
                    tt(w[k][:], w[k][:], mtile[:], AOT.mult)

                # keys + hash (f32 exact, mod 2^20)
                ges = {}
                for i in range(3):
                    for th in (1, 2, 3):
                        g = TT(f"ge{i}{th}")
                        ts(g[:], r[i][:], float(th), AOT.is_ge)
                        ges[(i, th)] = g

                def mod_pow2(dst, src, p2, tmp):
                    # dst = src - p2*floor(src/p2); |src| < 2^22, p2 power of two
                    ts(tmp[:], src[:], 1.0 / p2, AOT.mult)
                    f_round(dst, tmp[:])
                    tt(t4[:], dst[:], tmp[:], AOT.is_gt)
                    tt(dst[:], dst[:], t4[:], AOT.subtract)        # floor
                    stt(dst[:], dst[:], -float(p2), AOT.mult, src[:], AOT.add)

                key = TT("key"); u = TT("u"); a = TT("a"); hsum = TT("hsum"); m10 = TT("m10")
                for k in range(4):
                    for i in range(3):
                        # key_ik = rem_i + k - 4*ge(rank_i, 4-k)   (k=0 -> rem_i)
                        if k == 0:
                            src = rem[i]
                        else:
                            stt(key[:], ges[(i, 4 - k)][:], -4.0, AOT.mult, rem[i][:], AOT.add)
                            ts(key[:], key[:], float(k), AOT.add)
                            src = key
                        Ah, Al = MULTS[i] // 1024, MULTS[i] % 1024
                        ts(u[:], src[:], float(Ah), AOT.mult)      # key*Ah  (exact, <2^20)
                        mod_pow2(m10, u, 1024.0, t1)               # (key*Ah) mod 1024
                        ts(a[:], src[:], float(Al), AOT.mult)      # key*Al  (exact)
                        stt(a[:], m10[:], 1024.0, AOT.mult, a[:], AOT.add)
                        if i == 0:
                            nc.vector.tensor_copy(hsum[:], a[:])
                        else:
                            tt(hsum[:], hsum[:], a[:], AOT.add)
                    slot = TT(f"slot{k}")
                    mod_pow2(slot, hsum, float(CAP), t1)

                    # transpose slot & w to point-major and store to resident
                    pt = hpsum.tile([128, 128], F32, tag="pt", space="PSUM", name="pt_a")
                    nc.tensor.transpose(out=pt[:], in_=slot[:], identity=identity[:])
                    nc.scalar.copy(slotT_f[k][:, h * 128 : (h + 1) * 128], pt[:])
                    if not cce:
                        nc.vector.tensor_copy(slotT_i[k][:, h * 128 : (h + 1) * 128], pt[:])
                    pt2 = hpsum.tile([128, 128], F32, tag="pt", space="PSUM", name="pt_b")
                    nc.tensor.transpose(out=pt2[:], in_=w[k][:], identity=identity[:])
                    nc.scalar.copy(wT[k][:, h * 128 : (h + 1) * 128], pt2[:])

            hctx.close()

            # ================= Phase S =================
            sctx = ExitStack()
            sp = sctx.enter_context(tc.tile_pool(name="sp", bufs=4))
            gp = sctx.enter_context(tc.tile_pool(name="gp", bufs=gather_bufs))
            spsum = sctx.enter_context(tc.tile_pool(name="spsum", bufs=1, space="PSUM"))

            vals_flat = vals.rearrange("n d -> (n d)")

            def chunk_body(iv):
                vt = sp.tile([128, 64], F32, tag="vt", name="vt" + '_7')
                nc.sync.dma_start(
                    vt[:],
                    vals_flat[bass.ds(iv * 8192, 8192)].rearrange("(p f) -> p f", p=128),
                )
                for k in range(4):
                    wcol = wT[k][:, bass.ds(iv, 1)]
                    rows = sp.tile([128, 65], F32, tag=f"rows{k}", name=f"rows{k}" + '_8')
                    tt(rows[:, 0:64], vt[:], wcol.to_broadcast([128, 64]), AOT.mult)
                    nc.vector.tensor_copy(rows[:, 64:65], wcol)

                    # selection matrix (copy dynamic column to fixed tile:
                    # PE ldweights cannot take register offsets)
                    scol = sp.tile([128, 1], F32, tag=f"scol{k}", name=f"scol{k}")
                    nc.vector.tensor_copy(scol[:], slotT_f[k][:, bass.ds(iv, 1)])
                    if not cce:
                        sicol = sp.tile([128, 1], I32, tag=f"sicol{k}", name=f"sicol{k}")
                        nc.vector.tensor_copy(sicol[:], slotT_i[k][:, bass.ds(iv, 1)])
                    srow = spsum.tile([128, 128], F32, tag=f"tp{k}", space="PSUM", name=f"srow{k}")
                    nc.tensor.transpose(
                        out=srow[:],
                        in_=scol[:].to_broadcast([128, 128]),
                        identity=identity[:],
                    )
                    sel = sp.tile([128, 128], F32, tag=f"sel{k}", name=f"sel{k}" + '_9')
                    tt(sel[:], scol[:].to_broadcast([128, 128]), srow[:], AOT.is_equal)

                    acc = spsum.tile([128, 65], F32, tag=f"acc{k}", space="PSUM", name=f"acc{k}")
                    nc.tensor.matmul(out=acc[:], lhsT=sel[:], rhs=rows[:], start=True, stop=True)

                    if cce:
                        # duplicate rows (their sums are already carried by the
                        # first occurrence) are routed to trash row CAP+p —
                        # always in-bounds, so no descriptor is ever dropped.
                        msel = sp.tile([128, 128], F32, tag=f"msel{k}", name=f"msel{k}")
                        cnt = sp.tile([128, 1], F32, tag=f"cnt{k}", name=f"cnt{k}")
                        nc.vector.tensor_tensor_reduce(
                            out=msel[:], in0=sel[:], in1=ltmask[:], scale=1.0,
                            scalar=0.0, op0=AOT.mult, op1=AOT.add, accum_out=cnt[:],
                        )
                        g = sp.tile([128, 1], F32, tag=f"g{k}", name=f"g{k}")
                        tt(g[:], cappt[:], scol[:], AOT.subtract)      # (CAP+p) - slot
                        offf = sp.tile([128, 1], F32, tag=f"offf{k}", name=f"offf{k}")
                        stt(offf[:], cnt[:], 0.0, AOT.is_gt, g[:], AOT.mult)
                        tt(offf[:], offf[:], scol[:], AOT.add)         # dup ? CAP+p : slot
                        oicol = sp.tile([128, 1], I32, tag=f"oic{k}", name=f"oic{k}")
                        nc.vector.tensor_copy(oicol[:], offf[:])
                        accs = gp.tile([128, 65], F32, tag=f"accs{k}", name=f"accs{k}")
                        nc.scalar.copy(accs[:], acc[:])
                        nc.gpsimd.indirect_dma_start(
                            out=tabs[k][:],
                            out_offset=bass.IndirectOffsetOnAxis(ap=oicol[:], axis=0),
                            in_=accs[:],
                            in_offset=None,
                            compute_op=AOT.add,
                        )
                    else:
                        cur = gp.tile([128, 65], F32, tag=f"cur{k}", name=f"cur{k}" + '_10')
                        nc.gpsimd.indirect_dma_start(
                            out=cur[:],
                            out_offset=None,
                            in_=tabs[k][:],
                            in_offset=bass.IndirectOffsetOnAxis(ap=sicol[:], axis=0),
                        )
                        new = gp.tile([128, 65], F32, tag=f"new{k}", name=f"new{k}" + '_11')
                        tt(new[:], cur[:], acc[:], AOT.add)
                        nc.gpsimd.indirect_dma_start(
                            out=tabs[k][:],
                            out_offset=bass.IndirectOffsetOnAxis(ap=sicol[:], axis=0),
                            in_=new[:],
                            in_offset=None,
                        )

            for _sr in range(s_reps):
                if loop_mode == "for_i_unrolled":
                    tc.For_i_unrolled(0, NCH, 1, chunk_body, max_unroll=unroll)
                elif loop_mode == "for_i":
                    with tc.For_i(0, NCH, 1) as _iv:
                        chunk_body(_iv)
                else:
                    for _t in range(NCH):
                        chunk_body(_t)

            sctx.close()

            # ================= Phase M =================
            mp = ctx.enter_context(tc.tile_pool(name="mp", bufs=2))
            MF = n_merge_free
            flat_out = out[0:CAP, :].rearrange("v d -> (v d)")
            flats = [t[0:CAP, :].rearrange("v d -> (v d)") for t in tabs]
            per_part = CAP * 65 // 128              # 532480
            nmt = per_part // MF
            mrem = per_part - nmt * MF
            out2d = flat_out.rearrange("(p f) -> p f", p=128)
            tabs2d = [f.rearrange("(p f) -> p f", p=128) for f in flats]
            for _mr in range(m_reps if do_merge else 0):
             for i in range(nmt + (1 if mrem else 0)):
                lo = i * MF
                hi = min((i + 1) * MF, per_part)
                w_ = hi - lo
                tin = [mp.tile([128, MF], F32, tag=f"min{_k}", name=f"min{_k}_12") for _k in range(4)]
                for k in range(4):
                    nc.sync.dma_start(tin[k][:, :w_], tabs2d[k][:, lo:hi])
                tt(tin[0][:, :w_], tin[0][:, :w_], tin[1][:, :w_], AOT.add)
                tt(tin[2][:, :w_], tin[2][:, :w_], tin[3][:, :w_], AOT.add)
                tout = mp.tile([128, MF], F32, tag="mout", name="mout" + '_13')
                tt(tout[:, :w_], tin[0][:, :w_], tin[2][:, :w_], AOT.add)
                nc.sync.dma_start(out2d[:, lo:hi], tout[:, :w_])

    return dict(NP=NP)


def build2(nc, NP, unroll=8, z_reps=1, h_reps=1, p_reps=1, s_reps=1, m_reps=1,
           merge_mode="device", zero_mode="device", pbufs=8, gbufs=8,
           stage_mode="dram", loop_mode="for_i_unrolled"):
    """v2: concatenated per-class table [4*CAP, 65] + product staging.

    Phase H: hash math -> resident slot/weight planes (as build()).
    Phase P: per chunk, dedup'd product rows [128, 4*65] -> DRAM staging
             (direct DMAs, no chains).
    Phase S: per chunk, ONE 512-row gather + add + ONE 512-row scatter on the
             concat table; the only serial dependency is the table itself.
    Phase M: out[slot] = sum_k ctab[k*CAP+slot] (device) or on host.
    """
    assert NP % 16384 == 0
    NT = NP // 16384
    NCH = NP // 128

    pos = nc.dram_tensor("positions", [NP * 3], F32, kind="ExternalInput").ap()
    vals = nc.dram_tensor("values", [NP, 64], F32, kind="ExternalInput").ap()
    msk = nc.dram_tensor("mask", [NP], F32, kind="ExternalInput").ap()
    ident = nc.dram_tensor("ident", [128, 128], F32, kind="ExternalInput").ap()

    out = nc.dram_tensor("out", [CAP, 65], F32, kind="ExternalOutput").ap()
    if merge_mode == "host":
        assert zero_mode == "donated"
        tab_kind = "ExternalOutput"
    else:
        assert zero_mode == "device"
        tab_kind = "Internal"
    tabs = [out] + [
        nc.dram_tensor(f"tab{k}", [CAP, 65], F32, kind=tab_kind).ap()
        for k in range(1, DP1)
    ]
    prod = (nc.dram_tensor("prod", [NCH * 128 * 260], F32, kind="Internal").ap()
            if stage_mode == "dram" else None)

    with tile.TileContext(nc) as tc:
        with ExitStack() as ctx:
            resident = ctx.enter_context(tc.tile_pool(name="resident", bufs=1))
            identity = resident.tile([128, 128], F32, tag="ident", name="ident2")
            nc.sync.dma_start(identity[:], ident[:])

            slotT_f = [resident.tile([128, NCH], F32, tag=f"sf{k}", name=f"v2sf{k}") for k in range(DP1)]
            slotC_i = [resident.tile([128, NCH], I32, tag=f"sc{k}", name=f"v2sc{k}") for k in range(DP1)]
            wT = [resident.tile([128, NCH], F32, tag=f"w{k}", name=f"v2w{k}") for k in range(DP1)]

            # ---- Phase Z: zero the tables ----
            if zero_mode == "device":
                zpool = ctx.enter_context(tc.tile_pool(name="zpool", bufs=1))
                ztile = zpool.tile([128, 4096], F32, name="ztile2")
                nc.vector.memset(ztile[:], 0.0)
                total = CAP * 65
                zchunk = 128 * 4096
                nzfull = total // zchunk
                zrem = total - nzfull * zchunk
                for _zr in range(z_reps):
                 for k in range(DP1):
                    flat = tabs[k].rearrange("v d -> (v d)")
                    for i in range(nzfull):
                        nc.sync.dma_start(
                            flat[i * zchunk : (i + 1) * zchunk].rearrange("(p f) -> p f", p=128),
                            ztile[:],
                        )
                    if zrem:
                        assert zrem % 128 == 0
                        nc.sync.dma_start(
                            flat[nzfull * zchunk :].rearrange("(p f) -> p f", p=128),
                            ztile[:, : zrem // 128],
                        )

            # ---- Phase H (same math as build(), different residents) ----
            hctx = ExitStack()
            hp = hctx.enter_context(tc.tile_pool(name="hash", bufs=2))
            hpsum = hctx.enter_context(tc.tile_pool(name="hpsum", bufs=4, space="PSUM"))

            def TT(tag):
                return hp.tile([128, 128], F32, tag=tag, name=tag + "_v2")

            def ts(out_, in_, s0, op0, s1=None, op1=None):
                if s1 is None:
                    nc.vector.tensor_scalar(out_, in_, s0, None, op0)
                else:
                    nc.vector.tensor_scalar(out_, in_, s0, s1, op0, op1)

            def tt(out_, a, b, op):
                nc.vector.tensor_tensor(out=out_, in0=a, in1=b, op=op)

            def stt(out_, in0, s, op0, in1, op1):
                nc.vector.scalar_tensor_tensor(out=out_, in0=in0, scalar=s, in1=in1, op0=op0, op1=op1)

            def f_round(dst, src):
                ts(dst, src, MAGIC, AOT.add)
                ts(dst, dst[:], MAGIC, AOT.subtract)

            for _hr in range(h_reps):
             for h in range(NT):
                ptile = hp.tile([128, 384], F32, tag="pos", name="pos_v2")
                nc.sync.dma_start(ptile[:], pos[h * 49152 : (h + 1) * 49152].rearrange("(p f) -> p f", p=128))
                p3 = ptile[:].rearrange("p (t c) -> p t c", c=3)

                c = [TT(f"c{i}") for i in range(3)]
                for i in range(3):
                    ts(c[i][:], p3[:, :, i], SCALES[i], AOT.mult)

                e = [TT(f"e{i}") for i in range(4)]
                tt(e[1][:], c[1][:], c[2][:], AOT.add)
                tt(e[0][:], c[0][:], e[1][:], AOT.add)
                tt(e[1][:], e[1][:], c[0][:], AOT.subtract)
                stt(e[2][:], c[1][:], -2.0, AOT.mult, c[2][:], AOT.add)
                ts(e[3][:], c[2][:], -3.0, AOT.mult)

                rem = [TT(f"rem{i}") for i in range(4)]
                dif = [TT(f"dif{i}") for i in range(4)]
                t1 = TT("t1"); t2 = TT("t2"); t3 = TT("t3"); t4 = TT("t4")
                for i in range(4):
                    ts(t1[:], e[i][:], 0.25, AOT.mult)
                    f_round(t2[:], t1[:])
                    tt(t3[:], t2[:], t1[:], AOT.is_gt)
                    tt(t3[:], t2[:], t3[:], AOT.subtract)
                    tt(t4[:], t2[:], t1[:], AOT.is_lt)
                    tt(t4[:], t2[:], t4[:], AOT.add)
                    ts(t3[:], t3[:], 4.0, AOT.mult)
                    ts(t4[:], t4[:], 4.0, AOT.mult)
                    tt(t2[:], t4[:], e[i][:], AOT.subtract)
                    tt(t1[:], e[i][:], t3[:], AOT.subtract)
                    tt(t2[:], t2[:], t1[:], AOT.is_lt)
                    stt(rem[i][:], t2[:], 4.0, AOT.mult, t3[:], AOT.add)
                    tt(dif[i][:], e[i][:], rem[i][:], AOT.subtract)

                lt = {}
                for i in range(4):
                    for j in range(i + 1, 4):
                        lt[(i, j)] = TT(f"lt{i}{j}")
                        tt(lt[(i, j)][:], dif[i][:], dif[j][:], AOT.is_lt)
                r = [TT(f"r{i}") for i in range(4)]
                tt(r[0][:], lt[(0, 1)][:], lt[(0, 2)][:], AOT.add)
                tt(r[0][:], r[0][:], lt[(0, 3)][:], AOT.add)
                tt(r[1][:], lt[(1, 2)][:], lt[(1, 3)][:], AOT.add)
                ts(t1[:], lt[(0, 1)][:], -1.0, AOT.mult, 1.0, AOT.add)
                tt(r[1][:], r[1][:], t1[:], AOT.add)
                ts(t1[:], lt[(0, 2)][:], -1.0, AOT.mult, 2.0, AOT.add)
                tt(t1[:], t1[:], lt[(1, 2)][:], AOT.subtract)
                tt(r[2][:], t1[:], lt[(2, 3)][:], AOT.add)
                tt(t1[:], lt[(0, 3)][:], lt[(1, 3)][:], AOT.add)
                tt(t1[:], t1[:], lt[(2, 3)][:], AOT.add)
                ts(r[3][:], t1[:], -1.0, AOT.mult, 3.0, AOT.add)

                tt(t1[:], rem[0][:], rem[1][:], AOT.add)
                tt(t1[:], t1[:], rem[2][:], AOT.add)
                tt(t1[:], t1[:], rem[3][:], AOT.add)
                ts(t1[:], t1[:], 0.25, AOT.mult)
                for i in range(4):
                    tt(r[i][:], r[i][:], t1[:], AOT.add)
                for i in range(4):
                    ts(t2[:], r[i][:], 0.0, AOT.is_lt)
                    ts(t3[:], r[i][:], 3.0, AOT.is_gt)
                    stt(rem[i][:], t2[:], 4.0, AOT.mult, rem[i][:], AOT.add)
                    stt(rem[i][:], t3[:], -4.0, AOT.mult, rem[i][:], AOT.add)
                    stt(r[i][:], t2[:], 4.0, AOT.mult, r[i][:], AOT.add)
                    stt(r[i][:], t3[:], -4.0, AOT.mult, r[i][:], AOT.add)

                delta = [TT(f"dl{i}") for i in range(4)]
                for i in range(4):
                    tt(delta[i][:], e[i][:], rem[i][:], AOT.subtract)
                    ts(delta[i][:], delta[i][:], 0.25, AOT.mult)

                sels = []
                for rv in range(4):
                    acc = TT(f"sel{rv}")
                    for i in range(4):
                        ts(t1[:], r[i][:], float(rv), AOT.is_equal)
                        tt(t1[:], t1[:], delta[i][:], AOT.mult)
                        if i == 0:
                            nc.vector.tensor_copy(acc[:], t1[:])
                        else:
                            tt(acc[:], acc[:], t1[:], AOT.add)
                    sels.append(acc)
                mtile = hp.tile([128, 128], F32, tag="msk", name="msk_v2")
                nc.sync.dma_start(mtile[:], msk[h * 16384 : (h + 1) * 16384].rearrange("(p f) -> p f", p=128))
                w = [TT(f"wv{k}") for k in range(4)]
                ts(t1[:], sels[0][:], -1.0, AOT.mult, 1.0, AOT.add)
                tt(w[0][:], sels[3][:], t1[:], AOT.add)
                tt(w[1][:], sels[2][:], sels[3][:], AOT.subtract)
                tt(w[2][:], sels[1][:], sels[2][:], AOT.subtract)
                tt(w[3][:], sels[0][:], sels[1][:], AOT.subtract)
                for k in range(4):
                    tt(w[k][:], w[k][:], mtile[:], AOT.mult)

                ges = {}
                for i in range(3):
                    for th in (1, 2, 3):
                        g = TT(f"ge{i}{th}")
                        ts(g[:], r[i][:], float(th), AOT.is_ge)
                        ges[(i, th)] = g

                def mod_pow2(dst, src, p2, tmp):
                    ts(tmp[:], src[:], 1.0 / p2, AOT.mult)
                    f_round(dst, tmp[:])
                    tt(t4[:], dst[:], tmp[:], AOT.is_gt)
                    tt(dst[:], dst[:], t4[:], AOT.subtract)
                    stt(dst[:], dst[:], -float(p2), AOT.mult, src[:], AOT.add)

                key = TT("key"); u = TT("u"); a = TT("a"); hsum = TT("hsum"); m10 = TT("m10")
                for k in range(4):
                    for i in range(3):
                        if k == 0:
                            src = rem[i]
                        else:
                            stt(key[:], ges[(i, 4 - k)][:], -4.0, AOT.mult, rem[i][:], AOT.add)
                            ts(key[:], key[:], float(k), AOT.add)
                            src = key
                        Ah, Al = MULTS[i] // 1024, MULTS[i] % 1024
                        ts(u[:], src[:], float(Ah), AOT.mult)
                        mod_pow2(m10, u, 1024.0, t1)
                        ts(a[:], src[:], float(Al), AOT.mult)
                        stt(a[:], m10[:], 1024.0, AOT.mult, a[:], AOT.add)
                        if i == 0:
                            nc.vector.tensor_copy(hsum[:], a[:])
                        else:
                            tt(hsum[:], hsum[:], a[:], AOT.add)
                    slot = TT(f"slot{k}")
                    mod_pow2(slot, hsum, float(CAP), t1)

                    pt = hpsum.tile([128, 128], F32, tag="pt", space="PSUM", name="pt_v2a")
                    nc.tensor.transpose(out=pt[:], in_=slot[:], identity=identity[:])
                    nc.scalar.copy(slotT_f[k][:, h * 128 : (h + 1) * 128], pt[:])
                    nc.vector.tensor_copy(slotC_i[k][:, h * 128 : (h + 1) * 128], pt[:])
                    pt2 = hpsum.tile([128, 128], F32, tag="pt", space="PSUM", name="pt_v2b")
                    nc.tensor.transpose(out=pt2[:], in_=w[k][:], identity=identity[:])
                    nc.scalar.copy(wT[k][:, h * 128 : (h + 1) * 128], pt2[:])

            hctx.close()

            # ---- Phase P: products -> DRAM staging ----
            pctx = ExitStack()
            pp = pctx.enter_context(tc.tile_pool(name="pp", bufs=pbufs))
            ppsum = pctx.enter_context(tc.tile_pool(name="ppsum", bufs=3, space="PSUM"))
            vals_flat = vals.rearrange("n d -> (n d)")
            if stage_mode == "fused":
                gpf = pctx.enter_context(tc.tile_pool(name="gpf", bufs=gbufs))

                def fused_body(iv):
                    vt = pp.tile([128, 64], F32, tag="vt", name="vt_v3")
                    nc.sync.dma_start(
                        vt[:],
                        vals_flat[bass.ds(iv * 8192, 8192)].rearrange("(p f) -> p f", p=128),
                    )
                    for k in range(4):
                        wcol = wT[k][:, bass.ds(iv, 1)]
                        rows = pp.tile([128, 65], F32, tag=f"rows{k}", name=f"rows{k}_v3")
                        tt(rows[:, 0:64], vt[:], wcol.to_broadcast([128, 64]), AOT.mult)
                        nc.vector.tensor_copy(rows[:, 64:65], wcol)

                        scol = pp.tile([128, 1], F32, tag=f"scol{k}", name=f"scol{k}_v3")
                        nc.vector.tensor_copy(scol[:], slotT_f[k][:, bass.ds(iv, 1)])
                        srow = ppsum.tile([128, 128], F32, tag="tp", space="PSUM", name=f"srow{k}_v3")
                        nc.tensor.transpose(
                            out=srow[:],
                            in_=scol[:].to_broadcast([128, 128]),
                            identity=identity[:],
                        )
                        sel = pp.tile([128, 128], F32, tag=f"sel{k}", name=f"sel{k}_v3")
                        tt(sel[:], scol[:].to_broadcast([128, 128]), srow[:], AOT.is_equal)

                        acc = ppsum.tile([128, 65], F32, tag="acc", space="PSUM", name=f"acc{k}_v3")
                        nc.tensor.matmul(out=acc[:], lhsT=sel[:], rhs=rows[:], start=True, stop=True)

                        offc = gpf.tile([128, 1], I32, tag=f"off{k}", name=f"off{k}_v3")
                        nc.vector.tensor_copy(offc[:], slotC_i[k][:, bass.ds(iv, 1)])
                        cur = gpf.tile([128, 65], F32, tag=f"cur{k}", name=f"cur{k}_v3")
                        nc.gpsimd.indirect_dma_start(
                            out=cur[:],
                            out_offset=None,
                            in_=tabs[k][:],
                            in_offset=bass.IndirectOffsetOnAxis(ap=offc[:], axis=0),
                        )
                        new = gpf.tile([128, 65], F32, tag=f"new{k}", name=f"new{k}_v3")
                        tt(new[:], cur[:], acc[:], AOT.add)
                        nc.gpsimd.indirect_dma_start(
                            out=tabs[k][:],
                            out_offset=bass.IndirectOffsetOnAxis(ap=offc[:], axis=0),
                            in_=new[:],
                            in_offset=None,
                        )

                for _sr in range(s_reps):
                    if loop_mode == "python":
                        for _t in range(NCH):
                            fused_body(_t)
                    else:
                        tc.For_i_unrolled(0, NCH, 1, fused_body, max_unroll=unroll)
                pctx.close()

            def prod_body(iv):
                assert stage_mode == "dram"
                vt = pp.tile([128, 64], F32, tag="vt", name="vt_v2")
                nc.sync.dma_start(
                    vt[:],
                    vals_flat[bass.ds(iv * 8192, 8192)].rearrange("(p f) -> p f", p=128),
                )
                prodt = pp.tile([128, 260], F32, tag="prodt", name="prodt_v2")
                for k in range(4):
                    wcol = wT[k][:, bass.ds(iv, 1)]
                    rows = pp.tile([128, 65], F32, tag=f"rows{k}", name=f"rows{k}_v2")
                    tt(rows[:, 0:64], vt[:], wcol.to_broadcast([128, 64]), AOT.mult)
                    nc.vector.tensor_copy(rows[:, 64:65], wcol)

                    scol = pp.tile([128, 1], F32, tag=f"scol{k}", name=f"scol{k}_v2")
                    nc.vector.tensor_copy(scol[:], slotT_f[k][:, bass.ds(iv, 1)])
                    srow = ppsum.tile([128, 128], F32, tag="tp", space="PSUM", name=f"srow{k}_v2")
                    nc.tensor.transpose(
                        out=srow[:],
                        in_=scol[:].to_broadcast([128, 128]),
                        identity=identity[:],
                    )
                    sel = pp.tile([128, 128], F32, tag=f"sel{k}", name=f"sel{k}_v2")
                    tt(sel[:], scol[:].to_broadcast([128, 128]), srow[:], AOT.is_equal)

                    acc = ppsum.tile([128, 65], F32, tag="acc", space="PSUM", name=f"acc{k}_v2")
                    nc.tensor.matmul(out=acc[:], lhsT=sel[:], rhs=rows[:], start=True, stop=True)
                    nc.scalar.copy(prodt[:, k * 65 : (k + 1) * 65], acc[:])

                nc.sync.dma_start(
                    prod[bass.ds(iv * 33280, 33280)].rearrange("(p f) -> p f", p=128),
                    prodt[:],
                )

            if stage_mode == "dram":
                for _pr in range(p_reps):
                    if loop_mode == "python":
                        for _t in range(NCH):
                            prod_body(_t)
                    else:
                        tc.For_i_unrolled(0, NCH, 1, prod_body, max_unroll=unroll)
                pctx.close()

            # ---- Phase S: gather-add-scatter chains on the 4 tables ----
            sctx = ExitStack()
            if stage_mode == "dram":
                gp = sctx.enter_context(tc.tile_pool(name="gp2", bufs=gbufs))

            def scat_body(iv):
                pt_ = gp.tile([128, 260], F32, tag="pl", name="pl_v2")
                nc.sync.dma_start(
                    pt_[:],
                    prod[bass.ds(iv * 33280, 33280)].rearrange("(p f) -> p f", p=128),
                )
                for k in range(4):
                    offc = gp.tile([128, 1], I32, tag=f"off{k}", name=f"off{k}_v2")
                    nc.vector.tensor_copy(offc[:], slotC_i[k][:, bass.ds(iv, 1)])
                    cur = gp.tile([128, 65], F32, tag=f"cur{k}", name=f"cur{k}_v2")
                    nc.gpsimd.indirect_dma_start(
                        out=cur[:],
                        out_offset=None,
                        in_=tabs[k][:],
                        in_offset=bass.IndirectOffsetOnAxis(ap=offc[:], axis=0),
                    )
                    new = gp.tile([128, 65], F32, tag=f"new{k}", name=f"new{k}_v2")
                    tt(new[:], cur[:], pt_[:, k * 65 : (k + 1) * 65], AOT.add)
                    nc.gpsimd.indirect_dma_start(
                        out=tabs[k][:],
                        out_offset=bass.IndirectOffsetOnAxis(ap=offc[:], axis=0),
                        in_=new[:],
                        in_offset=None,
                    )

            if stage_mode == "dram":
                for _sr in range(s_reps):
                    if loop_mode == "python":
                        for _t in range(NCH):
                            scat_body(_t)
                    else:
                        tc.For_i_unrolled(0, NCH, 1, scat_body, max_unroll=unroll)
            sctx.close()

            # ---- Phase M: fold the 4 class tables into out ----
            if merge_mode == "device":
                mp = ctx.enter_context(tc.tile_pool(name="mp", bufs=2))
                MF = 1024
                flat_out = out.rearrange("v d -> (v d)")
                per_part = CAP * 65 // 128
                nmt = per_part // MF
                mrem = per_part - nmt * MF
                out2d = flat_out.rearrange("(p f) -> p f", p=128)
                tabs2d = [
                    t.rearrange("v d -> (v d)").rearrange("(p f) -> p f", p=128)
                    for t in tabs
                ]
                for _mr in range(m_reps):
                 for i in range(nmt + (1 if mrem else 0)):
                    lo = i * MF
                    hi = min((i + 1) * MF, per_part)
                    w_ = hi - lo
                    tin = [mp.tile([128, MF], F32, tag=f"min{_k}", name=f"min{_k}_v2") for _k in range(4)]
                    for k in range(4):
                        nc.sync.dma_start(tin[k][:, :w_], tabs2d[k][:, lo:hi])
                    tt(tin[0][:, :w_], tin[0][:, :w_], tin[1][:, :w_], AOT.add)
                    tt(tin[2][:, :w_], tin[2][:, :w_], tin[3][:, :w_], AOT.add)
                    tout = mp.tile([128, MF], F32, tag="mout", name="mout_v2")
                    tt(tout[:, :w_], tin[0][:, :w_], tin[2][:, :w_], AOT.add)
                    nc.sync.dma_start(out2d[:, lo:hi], tout[:, :w_])


def make_core_inputs(pos_shard, val_shard, NP):
    """Pad a core's shard to NP points and build the input map."""
    n = pos_shard.shape[0]
    assert n <= NP
    pos = np.zeros((NP, 3), np.float32)
    pos[:n] = pos_shard
    valp = np.zeros((NP, 64), np.float32)
    valp[:n] = val_shard
    m = np.zeros((NP,), np.float32)
    m[:n] = 1.0
    return {
        "positions": pos.reshape(-1),
        "values": valp,
        "mask": m,
        "ident": np.eye(128, dtype=np.float32),
        "ltm": np.tril(np.ones((128, 128), np.float32), -1),
        "capp": (CAP + np.arange(128, dtype=np.float32)).reshape(128, 1),
    }


from concourse.bass_utils import run_bass_kernel_spmd

N_CORES = 8
_CACHE = {}

# Active kernel configuration ("builder" selects build()/build2()).
CONFIG = dict(builder="v2", merge_mode="device", zero_mode="device")


def build_cfg(nc, NP, cfg):
    cfg = dict(cfg)
    b = cfg.pop("builder", "v1")
    if b == "v2":
        build2(nc, NP, **cfg)
    else:
        build(nc, NP, **cfg)


def _get_program(NP):
    key = (NP, tuple(sorted(CONFIG.items())))
    if key not in _CACHE:
        nc = bacc.Bacc("TRN2", target_bir_lowering=False, debug=False, num_devices=N_CORES)
        build_cfg(nc, NP, CONFIG)
        nc.compile()
        _CACHE[key] = nc
    return _CACHE[key]


def kernel(positions, values, hash_capacity):
    positions = np.ascontiguousarray(np.asarray(positions, dtype=np.float32))
    values = np.ascontiguousarray(np.asarray(values, dtype=np.float32))
    assert int(hash_capacity) == CAP, f"kernel compiled for capacity {CAP}"
    n = positions.shape[0]
    nsh = (n + N_CORES - 1) // N_CORES
    NP = ((nsh + 16383) // 16384) * 16384

    nc = _get_program(NP)

    in_maps = []
    for c in range(N_CORES):
        lo, hi = c * nsh, min((c + 1) * nsh, n)
        in_maps.append(
            make_core_inputs(positions[lo:hi], values[lo:hi], NP)
        )

    res = run_bass_kernel_spmd(nc, in_maps, core_ids=list(range(N_CORES)))

    acc = np.zeros((CAP, 65), np.float64)
    for c in range(N_CORES):
        r = res.results[c]
        if "out" in r and r["out"].shape[0] >= CAP and r["out"].shape[0] < 2 * CAP:
            acc += r["out"][:CAP].astype(np.float64)
        else:  # concat table [4*CAP(+..), 65]: fold classes on host
            o = r["out"]
            for k in range(4):
                acc += o[k * CAP : (k + 1) * CAP].astype(np.float64)
        for nm in r:
            if nm.startswith("tab"):
                acc += r[nm][:CAP].astype(np.float64)
    return np.ascontiguousarray(acc.astype(np.float32))



# revision 27
# speedup vs baseline: 2.5938x; 2.5938x over previous
"""Distributed permutohedral-lattice splat (scatter-add) for 8 Trainium2 cores.

Strategy (data-parallel over points, per the sharding hint):
  - Each of the 8 NeuronCores gets 1/8 of the points (padded + masked).
  - On-core: the permutohedral slot/weight math runs in f32 on the vector
    engine (op-for-op mirror of the reference, incl. the uint32 hash done in
    exact-f32 limb arithmetic mod 2^20), laid out free-major
    [128 lanes x 128 points] and PE-transposed to point-major.
  - The scatter-add runs as 4 independent serial gather-combine-scatter
    chains (chain k = simplex vertex k) into 4 per-core partial tables.
    Within a 128-row chunk, duplicate slots are merged with a selection-
    matrix matmul (rows with equal slots all receive the full sum, so
    colliding DMA writes are identical); across chunks a chain is
    serialized by the table RAW/WAW dependency; across chains the tables
    are disjoint, so no ordering is needed.
  - The 4 partial tables are summed on-device; the 8 per-core tables are
    summed on the host (the all-reduce step of the hint, folded into the
    unshard step).
"""

import os
os.environ["NEURON_SCRATCHPAD_PAGE_SIZE"] = "2048"
import numpy as np
from contextlib import ExitStack

import concourse.bass as bass
import concourse.tile as tile
from concourse import bacc, mybir
from concourse._compat import with_exitstack

F32 = mybir.dt.float32
I32 = mybir.dt.int32
AOT = mybir.AluOpType

D = 3
DP1 = 4
CAP = 1 << 20
MAGIC = 12582912.0            # 1.5 * 2^23 : round-to-nearest-even trick for |x| < 2^22
HMUL = 2531011
C20 = HMUL % CAP
B20 = (HMUL * HMUL) % (1 << 32) % CAP
A20 = ((HMUL * HMUL) % (1 << 32)) * HMUL % (1 << 32) % CAP
MULTS = [A20, B20, C20]       # slot = (k0*A20 + k1*B20 + k2*C20) mod 2^20
SCALES = [float(np.float32(np.sqrt(2.0 / 3.0) * DP1 / np.sqrt((i + 1.0) * (i + 2.0)))) for i in range(D)]


def build(nc, NP, n_merge_free=1024, unroll=8, gather_bufs=2, loop_mode="for_i_unrolled", z_reps=1, h_reps=1, s_reps=1, m_reps=1, cce=False, tabs_external=False, do_zero=True, do_merge=True):
    """NP must be a multiple of 16384. Returns nothing; program built into nc."""
    assert NP % 16384 == 0
    if not do_merge or not do_zero:
        # un-merged partial tables must be returned to the host for the final
        # sum; un-zeroed tables rely on run_bass_kernel_spmd's donated
        # zero-initialized ExternalOutput buffers.
        assert tabs_external
    NT = NP // 16384              # hash tiles
    NCH = NP // 128               # point-chunks (columns in slotT/wT)

    pos = nc.dram_tensor("positions", [NP * 3], F32, kind="ExternalInput").ap()
    vals = nc.dram_tensor("values", [NP, 64], F32, kind="ExternalInput").ap()
    msk = nc.dram_tensor("mask", [NP], F32, kind="ExternalInput").ap()
    ident = nc.dram_tensor("ident", [128, 128], F32, kind="ExternalInput").ap()
    ltm = nc.dram_tensor("ltm", [128, 128], F32, kind="ExternalInput").ap()
    # cce mode: +128 trash rows per table — within-chunk duplicate rows are
    # scattered to row CAP+partition instead of being OOB-dropped (descriptor
    # drops upset DMA completion accounting on HW).
    TR = 128 if cce else 0
    out = nc.dram_tensor("out", [CAP + TR, 65], F32, kind="ExternalOutput").ap()
    tab_kind = "ExternalOutput" if tabs_external else "Internal"
    tabs = [out] + [
        nc.dram_tensor(f"tab{k}", [CAP + TR, 65], F32, kind=tab_kind).ap()
        for k in range(1, DP1)
    ]
    if cce:
        capp = nc.dram_tensor("capp", [128, 1], F32, kind="ExternalInput").ap()

    with tile.TileContext(nc) as tc:
        with ExitStack() as ctx:
            resident = ctx.enter_context(tc.tile_pool(name="resident", bufs=1))
            identity = resident.tile([128, 128], F32, tag="ident", name="ident" + '_1')
            nc.sync.dma_start(identity[:], ident[:])
            ltmask = resident.tile([128, 128], F32, tag="ltm", name="ltm")
            nc.sync.dma_start(ltmask[:], ltm[:])
            if cce:
                cappt = resident.tile([128, 1], F32, tag="capp", name="cappt")
                nc.sync.dma_start(cappt[:], capp[:])

            slotT_f = [resident.tile([128, NCH], F32, tag=f"sf{k}", name=f"sf{k}" + '_2') for k in range(DP1)]
            slotT_i = [resident.tile([128, NCH], I32, tag=f"si{k}", name=f"si{k}" + '_3') for k in range(DP1)] if not cce else None
            wT = [resident.tile([128, NCH], F32, tag=f"w{k}", name=f"w{k}" + '_4') for k in range(DP1)]

            # ---- memset all tables (incl. out: no reliance on harness zero-init) ----
            zpool = ctx.enter_context(tc.tile_pool(name="zpool", bufs=1))
            ztile = zpool.tile([128, 4096], F32, name="ztile")
            nc.vector.memset(ztile[:], 0.0)
            total = CAP * 65                      # f32 elements per table
            zchunk = 128 * 4096
            nzfull = total // zchunk              # 130 full chunks
            zrem = total - nzfull * zchunk        # remainder elements
            for _zr in range(z_reps if do_zero else 0):
             for k in range(0, DP1):
                flat = tabs[k][0:CAP, :].rearrange("v d -> (v d)")
                for i in range(nzfull):
                    nc.sync.dma_start(
                        flat[i * zchunk : (i + 1) * zchunk].rearrange("(p f) -> p f", p=128),
                        ztile[:],
                    )
                if zrem:
                    assert zrem % 128 == 0
                    nc.sync.dma_start(
                        flat[nzfull * zchunk :].rearrange("(p f) -> p f", p=128),
                        ztile[:, : zrem // 128],
                    )

            # ================= Phase H =================
            hctx = ExitStack()
            hp = hctx.enter_context(tc.tile_pool(name="hash", bufs=2))
            hpsum = hctx.enter_context(tc.tile_pool(name="hpsum", bufs=4, space="PSUM"))

            def TT(tag):
                return hp.tile([128, 128], F32, tag=tag, name=tag)

            def ts(out_, in_, s0, op0, s1=None, op1=None):
                if s1 is None:
                    nc.vector.tensor_scalar(out_, in_, s0, None, op0)
                else:
                    nc.vector.tensor_scalar(out_, in_, s0, s1, op0, op1)

            def tt(out_, a, b, op):
                nc.vector.tensor_tensor(out=out_, in0=a, in1=b, op=op)

            def stt(out_, in0, s, op0, in1, op1):
                nc.vector.scalar_tensor_tensor(out=out_, in0=in0, scalar=s, in1=in1, op0=op0, op1=op1)

            def f_round(dst, src):      # dst = rne(src), |src| < 2^22
                ts(dst, src, MAGIC, AOT.add)
                ts(dst, dst[:], MAGIC, AOT.subtract)

            for _hr in range(h_reps):
             for h in range(NT):
                ptile = hp.tile([128, 384], F32, tag="pos", name="pos" + '_5')
                nc.sync.dma_start(ptile[:], pos[h * 49152 : (h + 1) * 49152].rearrange("(p f) -> p f", p=128))
                p3 = ptile[:].rearrange("p (t c) -> p t c", c=3)

                c = [TT(f"c{i}") for i in range(3)]
                for i in range(3):
                    ts(c[i][:], p3[:, :, i], SCALES[i], AOT.mult)

                e = [TT(f"e{i}") for i in range(4)]
                # s2=c2; s1=c1+c2; s0=c0+s1; e=[s0, s1-c0, c2-2c1, -3c2]
                tt(e[1][:], c[1][:], c[2][:], AOT.add)            # e1 <- s1
                tt(e[0][:], c[0][:], e[1][:], AOT.add)            # e0 <- s0
                tt(e[1][:], e[1][:], c[0][:], AOT.subtract)       # e1 = s1 - c0
                stt(e[2][:], c[1][:], -2.0, AOT.mult, c[2][:], AOT.add)   # e2 = c2 - 2c1
                ts(e[3][:], c[2][:], -3.0, AOT.mult)              # e3 = -3c2

                rem = [TT(f"rem{i}") for i in range(4)]
                dif = [TT(f"dif{i}") for i in range(4)]
                t1 = TT("t1"); t2 = TT("t2"); t3 = TT("t3"); t4 = TT("t4")
                for i in range(4):
                    ts(t1[:], e[i][:], 0.25, AOT.mult)            # v
                    f_round(t2[:], t1[:])                          # tr
                    tt(t3[:], t2[:], t1[:], AOT.is_gt)            # tr > v
                    tt(t3[:], t2[:], t3[:], AOT.subtract)         # fl = tr - (tr>v)
                    tt(t4[:], t2[:], t1[:], AOT.is_lt)            # tr < v
                    tt(t4[:], t2[:], t4[:], AOT.add)              # ce = tr + (tr<v)
                    ts(t3[:], t3[:], 4.0, AOT.mult)               # down
                    ts(t4[:], t4[:], 4.0, AOT.mult)               # up
                    tt(t2[:], t4[:], e[i][:], AOT.subtract)       # up - e
                    tt(t1[:], e[i][:], t3[:], AOT.subtract)       # e - down
                    tt(t2[:], t2[:], t1[:], AOT.is_lt)            # pick up?
                    stt(rem[i][:], t2[:], 4.0, AOT.mult, t3[:], AOT.add)  # rem = down + 4*pick
                    tt(dif[i][:], e[i][:], rem[i][:], AOT.subtract)

                # ranks
                lt = {}
                for i in range(4):
                    for j in range(i + 1, 4):
                        lt[(i, j)] = TT(f"lt{i}{j}")
                        tt(lt[(i, j)][:], dif[i][:], dif[j][:], AOT.is_lt)
                r = [TT(f"r{i}") for i in range(4)]
                tt(r[0][:], lt[(0, 1)][:], lt[(0, 2)][:], AOT.add)
                tt(r[0][:], r[0][:], lt[(0, 3)][:], AOT.add)
                tt(r[1][:], lt[(1, 2)][:], lt[(1, 3)][:], AOT.add)
                ts(t1[:], lt[(0, 1)][:], -1.0, AOT.mult, 1.0, AOT.add)
                tt(r[1][:], r[1][:], t1[:], AOT.add)
                ts(t1[:], lt[(0, 2)][:], -1.0, AOT.mult, 2.0, AOT.add)
                tt(t1[:], t1[:], lt[(1, 2)][:], AOT.subtract)
                tt(r[2][:], t1[:], lt[(2, 3)][:], AOT.add)
                tt(t1[:], lt[(0, 3)][:], lt[(1, 3)][:], AOT.add)
                tt(t1[:], t1[:], lt[(2, 3)][:], AOT.add)
                ts(r[3][:], t1[:], -1.0, AOT.mult, 3.0, AOT.add)

                # sum_rem/4 ; shifts
                tt(t1[:], rem[0][:], rem[1][:], AOT.add)
                tt(t1[:], t1[:], rem[2][:], AOT.add)
                tt(t1[:], t1[:], rem[3][:], AOT.add)
                ts(t1[:], t1[:], 0.25, AOT.mult)                  # sum_rem
                for i in range(4):
                    tt(r[i][:], r[i][:], t1[:], AOT.add)
                for i in range(4):
                    ts(t2[:], r[i][:], 0.0, AOT.is_lt)            # rank < 0
                    ts(t3[:], r[i][:], 3.0, AOT.is_gt)            # rank > 3
                    stt(rem[i][:], t2[:], 4.0, AOT.mult, rem[i][:], AOT.add)
                    stt(rem[i][:], t3[:], -4.0, AOT.mult, rem[i][:], AOT.add)
                    stt(r[i][:], t2[:], 4.0, AOT.mult, r[i][:], AOT.add)
                    stt(r[i][:], t3[:], -4.0, AOT.mult, r[i][:], AOT.add)

                delta = [TT(f"dl{i}") for i in range(4)]
                for i in range(4):
                    tt(delta[i][:], e[i][:], rem[i][:], AOT.subtract)
                    ts(delta[i][:], delta[i][:], 0.25, AOT.mult)

                # weights: sel(r) = sum_i delta_i * (rank_i == r)
                sels = []
                for rv in range(4):
                    acc = TT(f"sel{rv}")
                    for i in range(4):
                        ts(t1[:], r[i][:], float(rv), AOT.is_equal)
                        tt(t1[:], t1[:], delta[i][:], AOT.mult)
                        if i == 0:
                            nc.vector.tensor_copy(acc[:], t1[:])
                        else:
                            tt(acc[:], acc[:], t1[:], AOT.add)
                    sels.append(acc)
                mtile = hp.tile([128, 128], F32, tag="msk", name="msk" + '_6')
                nc.sync.dma_start(mtile[:], msk[h * 16384 : (h + 1) * 16384].rearrange("(p f) -> p f", p=128))
                w = [TT(f"wv{k}") for k in range(4)]
                ts(t1[:], sels[0][:], -1.0, AOT.mult, 1.0, AOT.add)
                tt(w[0][:], sels[3][:], t1[:], AOT.add)
                tt(w[1][:], sels[2][:], sels[3][:], AOT.subtract)
                tt(w[2][:], sels[1][:], sels[2][:], AOT.subtract)
                tt(w[3][:], sels[0][:], sels[1][:], AOT.subtract)
                for k in range(4):
                    tt(w[k][:], w[k][:], mtile[:], AOT.mult)

                # keys + hash (f32 exact, mod 2^20)
                ges = {}
                for i in range(3):
                    for th in (1, 2, 3):
                        g = TT(f"ge{i}{th}")
                        ts(g[:], r[i][:], float(th), AOT.is_ge)
                        ges[(i, th)] = g

                def mod_pow2(dst, src, p2, tmp):
                    # dst = src - p2*floor(src/p2); |src| < 2^22, p2 power of two
                    ts(tmp[:], src[:], 1.0 / p2, AOT.mult)
                    f_round(dst, tmp[:])
                    tt(t4[:], dst[:], tmp[:], AOT.is_gt)
                    tt(dst[:], dst[:], t4[:], AOT.subtract)        # floor
                    stt(dst[:], dst[:], -float(p2), AOT.mult, src[:], AOT.add)

                key = TT("key"); u = TT("u"); a = TT("a"); hsum = TT("hsum"); m10 = TT("m10")
                for k in range(4):
                    for i in range(3):
                        # key_ik = rem_i + k - 4*ge(rank_i, 4-k)   (k=0 -> rem_i)
                        if k == 0:
                            src = rem[i]
                        else:
                            stt(key[:], ges[(i, 4 - k)][:], -4.0, AOT.mult, rem[i][:], AOT.add)
                            ts(key[:], key[:], float(k), AOT.add)
                            src = key
                        Ah, Al = MULTS[i] // 1024, MULTS[i] % 1024
                        ts(u[:], src[:], float(Ah), AOT.mult)      # key*Ah  (exact, <2^20)
                        mod_pow2(m10, u, 1024.0, t1)               # (key*Ah) mod 1024
                        ts(a[:], src[:], float(Al), AOT.mult)      # key*Al  (exact)
                        stt(a[:], m10[:], 1024.0, AOT.mult, a[:], AOT.add)
                        if i == 0:
                            nc.vector.tensor_copy(hsum[:], a[:])
                        else:
                            tt(hsum[:], hsum[:], a[:], AOT.add)
                    slot = TT(f"slot{k}")
                    mod_pow2(slot, hsum, float(CAP), t1)

                    # transpose slot & w to point-major and store to resident
                    pt = hpsum.tile([128, 128], F32, tag="pt", space="PSUM", name="pt_a")
                    nc.tensor.transpose(out=pt[:], in_=slot[:], identity=identity[:])
                    nc.scalar.copy(slotT_f[k][:, h * 128 : (h + 1) * 128], pt[:])
                    if not cce:
                        nc.vector.tensor_copy(slotT_i[k][:, h * 128 : (h + 1) * 128], pt[:])
                    pt2 = hpsum.tile([128, 128], F32, tag="pt", space="PSUM", name="pt_b")
                    nc.tensor.transpose(out=pt2[:], in_=w[k][:], identity=identity[:])
                    nc.scalar.copy(wT[k][:, h * 128 : (h + 1) * 128], pt2[:])

            hctx.close()

            # ================= Phase S =================
            sctx = ExitStack()
            sp = sctx.enter_context(tc.tile_pool(name="sp", bufs=4))
            gp = sctx.enter_context(tc.tile_pool(name="gp", bufs=gather_bufs))
            spsum = sctx.enter_context(tc.tile_pool(name="spsum", bufs=1, space="PSUM"))

            vals_flat = vals.rearrange("n d -> (n d)")

            def chunk_body(iv):
                vt = sp.tile([128, 64], F32, tag="vt", name="vt" + '_7')
                nc.sync.dma_start(
                    vt[:],
                    vals_flat[bass.ds(iv * 8192, 8192)].rearrange("(p f) -> p f", p=128),
                )
                for k in range(4):
                    wcol = wT[k][:, bass.ds(iv, 1)]
                    rows = sp.tile([128, 65], F32, tag=f"rows{k}", name=f"rows{k}" + '_8')
                    tt(rows[:, 0:64], vt[:], wcol.to_broadcast([128, 64]), AOT.mult)
                    nc.vector.tensor_copy(rows[:, 64:65], wcol)

                    # selection matrix (copy dynamic column to fixed tile:
                    # PE ldweights cannot take register offsets)
                    scol = sp.tile([128, 1], F32, tag=f"scol{k}", name=f"scol{k}")
                    nc.vector.tensor_copy(scol[:], slotT_f[k][:, bass.ds(iv, 1)])
                    if not cce:
                        sicol = sp.tile([128, 1], I32, tag=f"sicol{k}", name=f"sicol{k}")
                        nc.vector.tensor_copy(sicol[:], slotT_i[k][:, bass.ds(iv, 1)])
                    srow = spsum.tile([128, 128], F32, tag=f"tp{k}", space="PSUM", name=f"srow{k}")
                    nc.tensor.transpose(
                        out=srow[:],
                        in_=scol[:].to_broadcast([128, 128]),
                        identity=identity[:],
                    )
                    sel = sp.tile([128, 128], F32, tag=f"sel{k}", name=f"sel{k}" + '_9')
                    tt(sel[:], scol[:].to_broadcast([128, 128]), srow[:], AOT.is_equal)

                    acc = spsum.tile([128, 65], F32, tag=f"acc{k}", space="PSUM", name=f"acc{k}")
                    nc.tensor.matmul(out=acc[:], lhsT=sel[:], rhs=rows[:], start=True, stop=True)

                    if cce:
                        # duplicate rows (their sums are already carried by the
                        # first occurrence) are routed to trash row CAP+p —
                        # always in-bounds, so no descriptor is ever dropped.
                        msel = sp.tile([128, 128], F32, tag=f"msel{k}", name=f"msel{k}")
                        cnt = sp.tile([128, 1], F32, tag=f"cnt{k}", name=f"cnt{k}")
                        nc.vector.tensor_tensor_reduce(
                            out=msel[:], in0=sel[:], in1=ltmask[:], scale=1.0,
                            scalar=0.0, op0=AOT.mult, op1=AOT.add, accum_out=cnt[:],
                        )
                        g = sp.tile([128, 1], F32, tag=f"g{k}", name=f"g{k}")
                        tt(g[:], cappt[:], scol[:], AOT.subtract)      # (CAP+p) - slot
                        offf = sp.tile([128, 1], F32, tag=f"offf{k}", name=f"offf{k}")
                        stt(offf[:], cnt[:], 0.0, AOT.is_gt, g[:], AOT.mult)
                        tt(offf[:], offf[:], scol[:], AOT.add)         # dup ? CAP+p : slot
                        oicol = sp.tile([128, 1], I32, tag=f"oic{k}", name=f"oic{k}")
                        nc.vector.tensor_copy(oicol[:], offf[:])
                        accs = gp.tile([128, 65], F32, tag=f"accs{k}", name=f"accs{k}")
                        nc.scalar.copy(accs[:], acc[:])
                        nc.gpsimd.indirect_dma_start(
                            out=tabs[k][:],
                            out_offset=bass.IndirectOffsetOnAxis(ap=oicol[:], axis=0),
                            in_=accs[:],
                            in_offset=None,
                            compute_op=AOT.add,
                        )
                    else:
                        cur = gp.tile([128, 65], F32, tag=f"cur{k}", name=f"cur{k}" + '_10')
                        nc.gpsimd.indirect_dma_start(
                            out=cur[:],
                            out_offset=None,
                            in_=tabs[k][:],
                            in_offset=bass.IndirectOffsetOnAxis(ap=sicol[:], axis=0),
                        )
                        new = gp.tile([128, 65], F32, tag=f"new{k}", name=f"new{k}" + '_11')
                        tt(new[:], cur[:], acc[:], AOT.add)
                        nc.gpsimd.indirect_dma_start(
                            out=tabs[k][:],
                            out_offset=bass.IndirectOffsetOnAxis(ap=sicol[:], axis=0),
                            in_=new[:],
                            in_offset=None,
                        )

            for _sr in range(s_reps):
                if loop_mode == "for_i_unrolled":
                    tc.For_i_unrolled(0, NCH, 1, chunk_body, max_unroll=unroll)
                elif loop_mode == "for_i":
                    with tc.For_i(0, NCH, 1) as _iv:
                        chunk_body(_iv)
                else:
                    for _t in range(NCH):
                        chunk_body(_t)

            sctx.close()

            # ================= Phase M =================
            mp = ctx.enter_context(tc.tile_pool(name="mp", bufs=2))
            MF = n_merge_free
            flat_out = out[0:CAP, :].rearrange("v d -> (v d)")
            flats = [t[0:CAP, :].rearrange("v d -> (v d)") for t in tabs]
            per_part = CAP * 65 // 128              # 532480
            nmt = per_part // MF
            mrem = per_part - nmt * MF
            out2d = flat_out.rearrange("(p f) -> p f", p=128)
            tabs2d = [f.rearrange("(p f) -> p f", p=128) for f in flats]
            for _mr in range(m_reps if do_merge else 0):
             for i in range(nmt + (1 if mrem else 0)):
                lo = i * MF
                hi = min((i + 1) * MF, per_part)
                w_ = hi - lo
                tin = [mp.tile([128, MF], F32, tag=f"min{_k}", name=f"min{_k}_12") for _k in range(4)]
                for k in range(4):
                    nc.sync.dma_start(tin[k][:, :w_], tabs2d[k][:, lo:hi])
                tt(tin[0][:, :w_], tin[0][:, :w_], tin[1][:, :w_], AOT.add)
                tt(tin[2][:, :w_], tin[2][:, :w_], tin[3][:, :w_], AOT.add)
                tout = mp.tile([128, MF], F32, tag="mout", name="mout" + '_13')
                tt(tout[:, :w_], tin[0][:, :w_], tin[2][:, :w_], AOT.add)
                nc.sync.dma_start(out2d[:, lo:hi], tout[:, :w_])

    return dict(NP=NP)


def build2(nc, NP, unroll=8, z_reps=1, h_reps=1, p_reps=1, s_reps=1, m_reps=1,
           merge_mode="device", zero_mode="device", pbufs=8, gbufs=8,
           stage_mode="dram", loop_mode="for_i_unrolled"):
    """v2: concatenated per-class table [4*CAP, 65] + product staging.

    Phase H: hash math -> resident slot/weight planes (as build()).
    Phase P: per chunk, dedup'd product rows [128, 4*65] -> DRAM staging
             (direct DMAs, no chains).
    Phase S: per chunk, ONE 512-row gather + add + ONE 512-row scatter on the
             concat table; the only serial dependency is the table itself.
    Phase M: out[slot] = sum_k ctab[k*CAP+slot] (device) or on host.
    """
    assert NP % 16384 == 0
    NT = NP // 16384
    NCH = NP // 128

    pos = nc.dram_tensor("positions", [NP * 3], F32, kind="ExternalInput").ap()
    vals = nc.dram_tensor("values", [NP, 64], F32, kind="ExternalInput").ap()
    msk = nc.dram_tensor("mask", [NP], F32, kind="ExternalInput").ap()
    ident = nc.dram_tensor("ident", [128, 128], F32, kind="ExternalInput").ap()

    out = nc.dram_tensor("out", [CAP, 65], F32, kind="ExternalOutput").ap()
    if merge_mode == "host":
        assert zero_mode == "donated"
        tab_kind = "ExternalOutput"
    else:
        assert zero_mode == "device"
        tab_kind = "Internal"
    tabs = [out] + [
        nc.dram_tensor(f"tab{k}", [CAP, 65], F32, kind=tab_kind).ap()
        for k in range(1, DP1)
    ]
    prod = (nc.dram_tensor("prod", [NCH * 128 * 260], F32, kind="Internal").ap()
            if stage_mode == "dram" else None)

    with tile.TileContext(nc) as tc:
        with ExitStack() as ctx:
            resident = ctx.enter_context(tc.tile_pool(name="resident", bufs=1))
            identity = resident.tile([128, 128], F32, tag="ident", name="ident2")
            nc.sync.dma_start(identity[:], ident[:])

            slotT_f = [resident.tile([128, NCH], F32, tag=f"sf{k}", name=f"v2sf{k}") for k in range(DP1)]
            slotC_i = [resident.tile([128, NCH], I32, tag=f"sc{k}", name=f"v2sc{k}") for k in range(DP1)]
            wT = [resident.tile([128, NCH], F32, tag=f"w{k}", name=f"v2w{k}") for k in range(DP1)]

            # ---- Phase Z: zero the tables ----
            if zero_mode == "device":
                zpool = ctx.enter_context(tc.tile_pool(name="zpool", bufs=1))
                ztile = zpool.tile([128, 4096], F32, name="ztile2")
                nc.vector.memset(ztile[:], 0.0)
                total = CAP * 65
                zchunk = 128 * 4096
                nzfull = total // zchunk
                zrem = total - nzfull * zchunk
                for _zr in range(z_reps):
                 for k in range(DP1):
                    flat = tabs[k].rearrange("v d -> (v d)")
                    for i in range(nzfull):
                        nc.sync.dma_start(
                            flat[i * zchunk : (i + 1) * zchunk].rearrange("(p f) -> p f", p=128),
                            ztile[:],
                        )
                    if zrem:
                        assert zrem % 128 == 0
                        nc.sync.dma_start(
                            flat[nzfull * zchunk :].rearrange("(p f) -> p f", p=128),
                            ztile[:, : zrem // 128],
                        )

            # ---- Phase H (same math as build(), different residents) ----
            hctx = ExitStack()
            hp = hctx.enter_context(tc.tile_pool(name="hash", bufs=2))
            hpsum = hctx.enter_context(tc.tile_pool(name="hpsum", bufs=4, space="PSUM"))

            def TT(tag):
                return hp.tile([128, 128], F32, tag=tag, name=tag + "_v2")

            def ts(out_, in_, s0, op0, s1=None, op1=None):
                if s1 is None:
                    nc.vector.tensor_scalar(out_, in_, s0, None, op0)
                else:
                    nc.vector.tensor_scalar(out_, in_, s0, s1, op0, op1)

            def tt(out_, a, b, op):
                nc.vector.tensor_tensor(out=out_, in0=a, in1=b, op=op)

            def stt(out_, in0, s, op0, in1, op1):
                nc.vector.scalar_tensor_tensor(out=out_, in0=in0, scalar=s, in1=in1, op0=op0, op1=op1)

            def f_round(dst, src):
                ts(dst, src, MAGIC, AOT.add)
                ts(dst, dst[:], MAGIC, AOT.subtract)

            for _hr in range(h_reps):
             for h in range(NT):
                ptile = hp.tile([128, 384], F32, tag="pos", name="pos_v2")
                nc.sync.dma_start(ptile[:], pos[h * 49152 : (h + 1) * 49152].rearrange("(p f) -> p f", p=128))
                p3 = ptile[:].rearrange("p (t c) -> p t c", c=3)

                c = [TT(f"c{i}") for i in range(3)]
                for i in range(3):
                    ts(c[i][:], p3[:, :, i], SCALES[i], AOT.mult)

                e = [TT(f"e{i}") for i in range(4)]
                tt(e[1][:], c[1][:], c[2][:], AOT.add)
                tt(e[0][:], c[0][:], e[1][:], AOT.add)
                tt(e[1][:], e[1][:], c[0][:], AOT.subtract)
                stt(e[2][:], c[1][:], -2.0, AOT.mult, c[2][:], AOT.add)
                ts(e[3][:], c[2][:], -3.0, AOT.mult)

                rem = [TT(f"rem{i}") for i in range(4)]
                dif = [TT(f"dif{i}") for i in range(4)]
                t1 = TT("t1"); t2 = TT("t2"); t3 = TT("t3"); t4 = TT("t4")
                for i in range(4):
                    ts(t1[:], e[i][:], 0.25, AOT.mult)
                    f_round(t2[:], t1[:])
                    tt(t3[:], t2[:], t1[:], AOT.is_gt)
                    tt(t3[:], t2[:], t3[:], AOT.subtract)
                    tt(t4[:], t2[:], t1[:], AOT.is_lt)
                    tt(t4[:], t2[:], t4[:], AOT.add)
                    ts(t3[:], t3[:], 4.0, AOT.mult)
                    ts(t4[:], t4[:], 4.0, AOT.mult)
                    tt(t2[:], t4[:], e[i][:], AOT.subtract)
                    tt(t1[:], e[i][:], t3[:], AOT.subtract)
                    tt(t2[:], t2[:], t1[:], AOT.is_lt)
                    stt(rem[i][:], t2[:], 4.0, AOT.mult, t3[:], AOT.add)
                    tt(dif[i][:], e[i][:], rem[i][:], AOT.subtract)

                lt = {}
                for i in range(4):
                    for j in range(i + 1, 4):
                        lt[(i, j)] = TT(f"lt{i}{j}")
                        tt(lt[(i, j)][:], dif[i][:], dif[j][:], AOT.is_lt)
                r = [TT(f"r{i}") for i in range(4)]
                tt(r[0][:], lt[(0, 1)][:], lt[(0, 2)][:], AOT.add)
                tt(r[0][:], r[0][:], lt[(0, 3)][:], AOT.add)
                tt(r[1][:], lt[(1, 2)][:], lt[(1, 3)][:], AOT.add)
                ts(t1[:], lt[(0, 1)][:], -1.0, AOT.mult, 1.0, AOT.add)
                tt(r[1][:], r[1][:], t1[:], AOT.add)
                ts(t1[:], lt[(0, 2)][:], -1.0, AOT.mult, 2.0, AOT.add)
                tt(t1[:], t1[:], lt[(1, 2)][:], AOT.subtract)
                tt(r[2][:], t1[:], lt[(2, 3)][:], AOT.add)
                tt(t1[:], lt[(0, 3)][:], lt[(1, 3)][:], AOT.add)
                tt(t1[:], t1[:], lt[(2, 3)][:], AOT.add)
                ts(r[3][:], t1[:], -1.0, AOT.mult, 3.0, AOT.add)

                tt(t1[:], rem[0][:], rem[1][:], AOT.add)
                tt(t1[:], t1[:], rem[2][:], AOT.add)
                tt(t1[:], t1[:], rem[3][:], AOT.add)
                ts(t1[:], t1[:], 0.25, AOT.mult)
                for i in range(4):
                    tt(r[i][:], r[i][:], t1[:], AOT.add)
                for i in range(4):
                    ts(t2[:], r[i][:], 0.0, AOT.is_lt)
                    ts(t3[:], r[i][:], 3.0, AOT.is_gt)
                    stt(rem[i][:], t2[:], 4.0, AOT.mult, rem[i][:], AOT.add)
                    stt(rem[i][:], t3[:], -4.0, AOT.mult, rem[i][:], AOT.add)
                    stt(r[i][:], t2[:], 4.0, AOT.mult, r[i][:], AOT.add)
                    stt(r[i][:], t3[:], -4.0, AOT.mult, r[i][:], AOT.add)

                delta = [TT(f"dl{i}") for i in range(4)]
                for i in range(4):
                    tt(delta[i][:], e[i][:], rem[i][:], AOT.subtract)
                    ts(delta[i][:], delta[i][:], 0.25, AOT.mult)

                sels = []
                for rv in range(4):
                    acc = TT(f"sel{rv}")
                    for i in range(4):
                        ts(t1[:], r[i][:], float(rv), AOT.is_equal)
                        tt(t1[:], t1[:], delta[i][:], AOT.mult)
                        if i == 0:
                            nc.vector.tensor_copy(acc[:], t1[:])
                        else:
                            tt(acc[:], acc[:], t1[:], AOT.add)
                    sels.append(acc)
                mtile = hp.tile([128, 128], F32, tag="msk", name="msk_v2")
                nc.sync.dma_start(mtile[:], msk[h * 16384 : (h + 1) * 16384].rearrange("(p f) -> p f", p=128))
                w = [TT(f"wv{k}") for k in range(4)]
                ts(t1[:], sels[0][:], -1.0, AOT.mult, 1.0, AOT.add)
                tt(w[0][:], sels[3][:], t1[:], AOT.add)
                tt(w[1][:], sels[2][:], sels[3][:], AOT.subtract)
                tt(w[2][:], sels[1][:], sels[2][:], AOT.subtract)
                tt(w[3][:], sels[0][:], sels[1][:], AOT.subtract)
                for k in range(4):
                    tt(w[k][:], w[k][:], mtile[:], AOT.mult)

                ges = {}
                for i in range(3):
                    for th in (1, 2, 3):
                        g = TT(f"ge{i}{th}")
                        ts(g[:], r[i][:], float(th), AOT.is_ge)
                        ges[(i, th)] = g

                def mod_pow2(dst, src, p2, tmp):
                    ts(tmp[:], src[:], 1.0 / p2, AOT.mult)
                    f_round(dst, tmp[:])
                    tt(t4[:], dst[:], tmp[:], AOT.is_gt)
                    tt(dst[:], dst[:], t4[:], AOT.subtract)
                    stt(dst[:], dst[:], -float(p2), AOT.mult, src[:], AOT.add)

                key = TT("key"); u = TT("u"); a = TT("a"); hsum = TT("hsum"); m10 = TT("m10")
                for k in range(4):
                    for i in range(3):
                        if k == 0:
                            src = rem[i]
                        else:
                            stt(key[:], ges[(i, 4 - k)][:], -4.0, AOT.mult, rem[i][:], AOT.add)
                            ts(key[:], key[:], float(k), AOT.add)
                            src = key
                        Ah, Al = MULTS[i] // 1024, MULTS[i] % 1024
                        ts(u[:], src[:], float(Ah), AOT.mult)
                        mod_pow2(m10, u, 1024.0, t1)
                        ts(a[:], src[:], float(Al), AOT.mult)
                        stt(a[:], m10[:], 1024.0, AOT.mult, a[:], AOT.add)
                        if i == 0:
                            nc.vector.tensor_copy(hsum[:], a[:])
                        else:
                            tt(hsum[:], hsum[:], a[:], AOT.add)
                    slot = TT(f"slot{k}")
                    mod_pow2(slot, hsum, float(CAP), t1)

                    pt = hpsum.tile([128, 128], F32, tag="pt", space="PSUM", name="pt_v2a")
                    nc.tensor.transpose(out=pt[:], in_=slot[:], identity=identity[:])
                    nc.scalar.copy(slotT_f[k][:, h * 128 : (h + 1) * 128], pt[:])
                    nc.vector.tensor_copy(slotC_i[k][:, h * 128 : (h + 1) * 128], pt[:])
                    pt2 = hpsum.tile([128, 128], F32, tag="pt", space="PSUM", name="pt_v2b")
                    nc.tensor.transpose(out=pt2[:], in_=w[k][:], identity=identity[:])
                    nc.scalar.copy(wT[k][:, h * 128 : (h + 1) * 128], pt2[:])

            hctx.close()

            # ---- Phase P: products -> DRAM staging ----
            pctx = ExitStack()
            pp = pctx.enter_context(tc.tile_pool(name="pp", bufs=pbufs))
            ppsum = pctx.enter_context(tc.tile_pool(name="ppsum", bufs=3, space="PSUM"))
            vals_flat = vals.rearrange("n d -> (n d)")
            if stage_mode == "fused":
                gpf = pctx.enter_context(tc.tile_pool(name="gpf", bufs=gbufs))

                def fused_body(iv):
                    vt = pp.tile([128, 64], F32, tag="vt", name="vt_v3")
                    nc.sync.dma_start(
                        vt[:],
                        vals_flat[bass.ds(iv * 8192, 8192)].rearrange("(p f) -> p f", p=128),
                    )
                    for k in range(4):
                        wcol = wT[k][:, bass.ds(iv, 1)]
                        rows = pp.tile([128, 65], F32, tag=f"rows{k}", name=f"rows{k}_v3")
                        tt(rows[:, 0:64], vt[:], wcol.to_broadcast([128, 64]), AOT.mult)
                        nc.vector.tensor_copy(rows[:, 64:65], wcol)

                        scol = pp.tile([128, 1], F32, tag=f"scol{k}", name=f"scol{k}_v3")
                        nc.vector.tensor_copy(scol[:], slotT_f[k][:, bass.ds(iv, 1)])
                        srow = ppsum.tile([128, 128], F32, tag="tp", space="PSUM", name=f"srow{k}_v3")
                        nc.tensor.transpose(
                            out=srow[:],
                            in_=scol[:].to_broadcast([128, 128]),
                            identity=identity[:],
                        )
                        sel = pp.tile([128, 128], F32, tag=f"sel{k}", name=f"sel{k}_v3")
                        tt(sel[:], scol[:].to_broadcast([128, 128]), srow[:], AOT.is_equal)

                        acc = ppsum.tile([128, 65], F32, tag="acc", space="PSUM", name=f"acc{k}_v3")
                        nc.tensor.matmul(out=acc[:], lhsT=sel[:], rhs=rows[:], start=True, stop=True)

                        offc = gpf.tile([128, 1], I32, tag=f"off{k}", name=f"off{k}_v3")
                        nc.vector.tensor_copy(offc[:], slotC_i[k][:, bass.ds(iv, 1)])
                        cur = gpf.tile([128, 65], F32, tag=f"cur{k}", name=f"cur{k}_v3")
                        nc.gpsimd.indirect_dma_start(
                            out=cur[:],
                            out_offset=None,
                            in_=tabs[k][:],
                            in_offset=bass.IndirectOffsetOnAxis(ap=offc[:], axis=0),
                        )
                        new = gpf.tile([128, 65], F32, tag=f"new{k}", name=f"new{k}_v3")
                        tt(new[:], cur[:], acc[:], AOT.add)
                        nc.gpsimd.indirect_dma_start(
                            out=tabs[k][:],
                            out_offset=bass.IndirectOffsetOnAxis(ap=offc[:], axis=0),
                            in_=new[:],
                            in_offset=None,
                        )

                for _sr in range(s_reps):
                    if loop_mode == "python":
                        for _t in range(NCH):
                            fused_body(_t)
                    else:
                        tc.For_i_unrolled(0, NCH, 1, fused_body, max_unroll=unroll)
                pctx.close()

            def prod_body(iv):
                assert stage_mode == "dram"
                vt = pp.tile([128, 64], F32, tag="vt", name="vt_v2")
                nc.sync.dma_start(
                    vt[:],
                    vals_flat[bass.ds(iv * 8192, 8192)].rearrange("(p f) -> p f", p=128),
                )
                prodt = pp.tile([128, 260], F32, tag="prodt", name="prodt_v2")
                for k in range(4):
                    wcol = wT[k][:, bass.ds(iv, 1)]
                    rows = pp.tile([128, 65], F32, tag=f"rows{k}", name=f"rows{k}_v2")
                    tt(rows[:, 0:64], vt[:], wcol.to_broadcast([128, 64]), AOT.mult)
                    nc.vector.tensor_copy(rows[:, 64:65], wcol)

                    scol = pp.tile([128, 1], F32, tag=f"scol{k}", name=f"scol{k}_v2")
                    nc.vector.tensor_copy(scol[:], slotT_f[k][:, bass.ds(iv, 1)])
                    srow = ppsum.tile([128, 128], F32, tag="tp", space="PSUM", name=f"srow{k}_v2")
                    nc.tensor.transpose(
                        out=srow[:],
                        in_=scol[:].to_broadcast([128, 128]),
                        identity=identity[:],
                    )
                    sel = pp.tile([128, 128], F32, tag=f"sel{k}", name=f"sel{k}_v2")
                    tt(sel[:], scol[:].to_broadcast([128, 128]), srow[:], AOT.is_equal)

                    acc = ppsum.tile([128, 65], F32, tag="acc", space="PSUM", name=f"acc{k}_v2")
                    nc.tensor.matmul(out=acc[:], lhsT=sel[:], rhs=rows[:], start=True, stop=True)
                    nc.scalar.copy(prodt[:, k * 65 : (k + 1) * 65], acc[:])

                nc.sync.dma_start(
                    prod[bass.ds(iv * 33280, 33280)].rearrange("(p f) -> p f", p=128),
                    prodt[:],
                )

            if stage_mode == "dram":
                for _pr in range(p_reps):
                    if loop_mode == "python":
                        for _t in range(NCH):
                            prod_body(_t)
                    else:
                        tc.For_i_unrolled(0, NCH, 1, prod_body, max_unroll=unroll)
                pctx.close()

            # ---- Phase S: gather-add-scatter chains on the 4 tables ----
            sctx = ExitStack()
            if stage_mode == "dram":
                gp = sctx.enter_context(tc.tile_pool(name="gp2", bufs=gbufs))

            def scat_body(iv):
                pt_ = gp.tile([128, 260], F32, tag="pl", name="pl_v2")
                nc.sync.dma_start(
                    pt_[:],
                    prod[bass.ds(iv * 33280, 33280)].rearrange("(p f) -> p f", p=128),
                )
                for k in range(4):
                    offc = gp.tile([128, 1], I32, tag=f"off{k}", name=f"off{k}_v2")
                    nc.vector.tensor_copy(offc[:], slotC_i[k][:, bass.ds(iv, 1)])
                    cur = gp.tile([128, 65], F32, tag=f"cur{k}", name=f"cur{k}_v2")
                    nc.gpsimd.indirect_dma_start(
                        out=cur[:],
                        out_offset=None,
                        in_=tabs[k][:],
                        in_offset=bass.IndirectOffsetOnAxis(ap=offc[:], axis=0),
                    )
                    new = gp.tile([128, 65], F32, tag=f"new{k}", name=f"new{k}_v2")
                    tt(new[:], cur[:], pt_[:, k * 65 : (k + 1) * 65], AOT.add)
                    nc.gpsimd.indirect_dma_start(
                        out=tabs[k][:],
                        out_offset=bass.IndirectOffsetOnAxis(ap=offc[:], axis=0),
                        in_=new[:],
                        in_offset=None,
                    )

            if stage_mode == "dram":
                for _sr in range(s_reps):
                    if loop_mode == "python":
                        for _t in range(NCH):
                            scat_body(_t)
                    else:
                        tc.For_i_unrolled(0, NCH, 1, scat_body, max_unroll=unroll)
            sctx.close()

            # ---- Phase M: fold the 4 class tables into out ----
            if merge_mode == "device":
                mp = ctx.enter_context(tc.tile_pool(name="mp", bufs=2))
                MF = 1024
                flat_out = out.rearrange("v d -> (v d)")
                per_part = CAP * 65 // 128
                nmt = per_part // MF
                mrem = per_part - nmt * MF
                out2d = flat_out.rearrange("(p f) -> p f", p=128)
                tabs2d = [
                    t.rearrange("v d -> (v d)").rearrange("(p f) -> p f", p=128)
                    for t in tabs
                ]
                for _mr in range(m_reps):
                 for i in range(nmt + (1 if mrem else 0)):
                    lo = i * MF
                    hi = min((i + 1) * MF, per_part)
                    w_ = hi - lo
                    tin = [mp.tile([128, MF], F32, tag=f"min{_k}", name=f"min{_k}_v2") for _k in range(4)]
                    for k in range(4):
                        nc.sync.dma_start(tin[k][:, :w_], tabs2d[k][:, lo:hi])
                    tt(tin[0][:, :w_], tin[0][:, :w_], tin[1][:, :w_], AOT.add)
                    tt(tin[2][:, :w_], tin[2][:, :w_], tin[3][:, :w_], AOT.add)
                    tout = mp.tile([128, MF], F32, tag="mout", name="mout_v2")
                    tt(tout[:, :w_], tin[0][:, :w_], tin[2][:, :w_], AOT.add)
                    nc.sync.dma_start(out2d[:, lo:hi], tout[:, :w_])


NB = 32                 # buckets (table windows of 32768 rows)
WIN = 32768             # window rows (int16-addressable for dma_scatter_add)
CBCK = 24576            # staging capacity per bucket (mean 16384 + pad + slack)
CBP = CBCK // 128       # payload rows per partition in the pass-2 load
CBI = CBCK // 16        # idx entries per partition (wrapped int16 layout)


def build4(nc, NP, kblk=32, z_reps=1, h_reps=1, t_reps=1, s_reps=1, c_reps=1,
           pay_split=1):
    """v4: radix-bucketed scatter via dma_scatter_add.

    Sweep 1 (per 16K-point tile, per simplex class): hash math (free-major,
      no transposes) + per-bucket running ranks via tensor_tensor_scan +
      partition-prefix matmul.
    Global: tiny serial scan accumulates per-(tile,class) bucket totals.
    Sweep 2: finish ranks, compute disjoint staging offsets, scatter
      (products, slot_low) rows into bucketed DRAM staging via batched
      indirect DMAs (rows disjoint by construction -> no dedup anywhere).
    Pass 2 (per bucket): contiguous load of staged rows + ONE dma_scatter_add
      (duplicate-safe, int16 idx) into a [32768, 128]-strided scratch window.
    Compact: scratch [CAP,128] -> out [CAP,65].
    """
    assert NP % 16384 == 0
    NT = NP // 16384
    TC = NT * DP1
    assert TC <= 128

    pos = nc.dram_tensor("positions", [NP * 3], F32, kind="ExternalInput").ap()
    vals = nc.dram_tensor("values", [NP, 64], F32, kind="ExternalInput").ap()
    msk = nc.dram_tensor("mask", [NP], F32, kind="ExternalInput").ap()
    ltu = nc.dram_tensor("ltu", [128, 128], F32, kind="ExternalInput").ap()

    out = nc.dram_tensor("out", [CAP, 65], F32, kind="ExternalOutput").ap()
    scratch = nc.dram_tensor("scratch", [CAP, 128], F32, kind="Internal").ap()
    stg_pay = [
        nc.dram_tensor(f"stg_pay{s}", [NB * CBCK, 65], F32, kind="Internal").ap()
        for s in range(pay_split)
    ]
    stg_idx = nc.dram_tensor("stg_idx", [NB * CBCK, 1], F32, kind="Internal").ap()

    AX = mybir.AxisListType

    with tile.TileContext(nc) as tc:
      with ExitStack() as ctx:
        resident = ctx.enter_context(tc.tile_pool(name="res4", bufs=1))
        ltut = resident.tile([128, 128], F32, tag="ltu", name="ltut")
        nc.sync.dma_start(ltut[:], ltu[:])
        iotaP = resident.tile([128, 1], F32, tag="iop", name="iotaP")
        nc.gpsimd.iota(iotaP[:], pattern=[[0, 1]], base=0, channel_multiplier=1,
                       allow_small_or_imprecise_dtypes=True)

        # ---- Phase Z: zero scratch, init stg_idx = -1 ----
        zctx = ExitStack()
        zpool = zctx.enter_context(tc.tile_pool(name="z4", bufs=1))
        ztile = zpool.tile([128, 4096], F32, name="zt4")
        nc.vector.memset(ztile[:], 0.0)
        mtile = zpool.tile([128, 4096], F32, name="mt4")
        nc.vector.memset(mtile[:], -1.0)
        for _zr in range(z_reps):
            sflat = scratch.rearrange("r c -> (r c)")
            zc = 128 * 4096
            for i in range(CAP * 128 // zc):
                nc.sync.dma_start(
                    sflat[i * zc : (i + 1) * zc].rearrange("(p f) -> p f", p=128),
                    ztile[:],
                )
            iflat = stg_idx.rearrange("r c -> (r c)")
            tot = NB * CBCK
            nfull = tot // zc
            for i in range(nfull):
                nc.sync.dma_start(
                    iflat[i * zc : (i + 1) * zc].rearrange("(p f) -> p f", p=128),
                    mtile[:],
                )
            rem = tot - nfull * zc
            if rem:
                nc.sync.dma_start(
                    iflat[nfull * zc :].rearrange("(p f) -> p f", p=128),
                    mtile[:, : rem // 128],
                )
            if pay_split > 1:
                # splits are summed in pass 2: unwritten rows must be zero
                for s in range(pay_split):
                    pflat = stg_pay[s].rearrange("r c -> (r c)")
                    ptot = NB * CBCK * 65
                    pful = ptot // zc
                    for i in range(pful):
                        nc.sync.dma_start(
                            pflat[i * zc : (i + 1) * zc].rearrange("(p f) -> p f", p=128),
                            ztile[:],
                        )
                    prem = ptot - pful * zc
                    if prem:
                        nc.sync.dma_start(
                            pflat[pful * zc :].rearrange("(p f) -> p f", p=128),
                            ztile[:, : prem // 128],
                        )

        zctx.close()

        # ---- residents for sweep1 -> sweep2 handoff ----
        mid = ExitStack()
        midp = mid.enter_context(tc.tile_pool(name="mid4", bufs=1))
        RI = [midp.tile([128, NT * 128], F32, tag=f"ri{k}", name=f"ri{k}") for k in range(DP1)]
        RBk = [midp.tile([128, NT * 128], F32, tag=f"rb{k}", name=f"rb{k}") for k in range(DP1)]
        RLOW = [midp.tile([128, NT * 128], F32, tag=f"rl{k}", name=f"rl{k}") for k in range(DP1)]
        RW = [midp.tile([128, NT * 128], F32, tag=f"rw{k}", name=f"rw{k}") for k in range(DP1)]
        PBres = midp.tile([128, TC * 32], F32, tag="pb", name="pbres")
        TOTf = midp.tile([1, TC * 32], F32, tag="totf", name="totf")
        GBf = midp.tile([1, TC * 32], F32, tag="gbf", name="gbf")
        GBrhs = midp.tile([128, 32], F32, tag="gbrhs", name="gbrhs")
        NTOTi = resident.tile([1, 32], I32, tag="ntoti", name="ntoti")
        e0 = resident.tile([128, 1], F32, tag="e0", name="e0_b4")
        nc.vector.tensor_scalar(e0[:], iotaP[:], 0.0, None, AOT.is_equal)
        onest = resident.tile([128, 128], F32, tag="ones", name="ones_b4")
        nc.vector.memset(onest[:], 1.0)

        def ts(out_, in_, s0, op0, s1=None, op1=None):
            if s1 is None:
                nc.vector.tensor_scalar(out_, in_, s0, None, op0)
            else:
                nc.vector.tensor_scalar(out_, in_, s0, s1, op0, op1)

        def tt(out_, a, b, op):
            nc.vector.tensor_tensor(out=out_, in0=a, in1=b, op=op)

        def stt(out_, in0, s, op0, in1, op1):
            nc.vector.scalar_tensor_tensor(out=out_, in0=in0, scalar=s, in1=in1, op0=op0, op1=op1)

        def f_round(dst, src):
            ts(dst, src, MAGIC, AOT.add)
            ts(dst, dst[:], MAGIC, AOT.subtract)

        # ---- Sweep 1: hash math + intra-tile ranks ----
        s1ctx = ExitStack()
        hp = s1ctx.enter_context(tc.tile_pool(name="h4", bufs=2))
        hpsum = s1ctx.enter_context(tc.tile_pool(name="h4ps", bufs=4, space="PSUM"))

        def TT(tag):
            return hp.tile([128, 128], F32, tag=tag, name=tag + "_b4")

        def mod_pow2(dst, src, p2, tmp, t4):
            ts(tmp[:], src[:], 1.0 / p2, AOT.mult)
            f_round(dst, tmp[:])
            tt(t4[:], dst[:], tmp[:], AOT.is_gt)
            tt(dst[:], dst[:], t4[:], AOT.subtract)
            stt(dst[:], dst[:], -float(p2), AOT.mult, src[:], AOT.add)

        for _hr in range(h_reps):
         for h in range(NT):
            ptile = hp.tile([128, 384], F32, tag="pos", name="pos_b4")
            nc.sync.dma_start(ptile[:], pos[h * 49152 : (h + 1) * 49152].rearrange("(p f) -> p f", p=128))
            p3 = ptile[:].rearrange("p (t c) -> p t c", c=3)

            c = [TT(f"c{i}") for i in range(3)]
            for i in range(3):
                ts(c[i][:], p3[:, :, i], SCALES[i], AOT.mult)

            e = [TT(f"e{i}") for i in range(4)]
            tt(e[1][:], c[1][:], c[2][:], AOT.add)
            tt(e[0][:], c[0][:], e[1][:], AOT.add)
            tt(e[1][:], e[1][:], c[0][:], AOT.subtract)
            stt(e[2][:], c[1][:], -2.0, AOT.mult, c[2][:], AOT.add)
            ts(e[3][:], c[2][:], -3.0, AOT.mult)

            rem = [TT(f"rem{i}") for i in range(4)]
            dif = [TT(f"dif{i}") for i in range(4)]
            t1 = TT("t1"); t2 = TT("t2"); t3 = TT("t3"); t4 = TT("t4")
            for i in range(4):
                ts(t1[:], e[i][:], 0.25, AOT.mult)
                f_round(t2[:], t1[:])
                tt(t3[:], t2[:], t1[:], AOT.is_gt)
                tt(t3[:], t2[:], t3[:], AOT.subtract)
                tt(t4[:], t2[:], t1[:], AOT.is_lt)
                tt(t4[:], t2[:], t4[:], AOT.add)
                ts(t3[:], t3[:], 4.0, AOT.mult)
                ts(t4[:], t4[:], 4.0, AOT.mult)
                tt(t2[:], t4[:], e[i][:], AOT.subtract)
                tt(t1[:], e[i][:], t3[:], AOT.subtract)
                tt(t2[:], t2[:], t1[:], AOT.is_lt)
                stt(rem[i][:], t2[:], 4.0, AOT.mult, t3[:], AOT.add)
                tt(dif[i][:], e[i][:], rem[i][:], AOT.subtract)

            lt = {}
            for i in range(4):
                for j in range(i + 1, 4):
                    lt[(i, j)] = TT(f"lt{i}{j}")
                    tt(lt[(i, j)][:], dif[i][:], dif[j][:], AOT.is_lt)
            r = [TT(f"r{i}") for i in range(4)]
            tt(r[0][:], lt[(0, 1)][:], lt[(0, 2)][:], AOT.add)
            tt(r[0][:], r[0][:], lt[(0, 3)][:], AOT.add)
            tt(r[1][:], lt[(1, 2)][:], lt[(1, 3)][:], AOT.add)
            ts(t1[:], lt[(0, 1)][:], -1.0, AOT.mult, 1.0, AOT.add)
            tt(r[1][:], r[1][:], t1[:], AOT.add)
            ts(t1[:], lt[(0, 2)][:], -1.0, AOT.mult, 2.0, AOT.add)
            tt(t1[:], t1[:], lt[(1, 2)][:], AOT.subtract)
            tt(r[2][:], t1[:], lt[(2, 3)][:], AOT.add)
            tt(t1[:], lt[(0, 3)][:], lt[(1, 3)][:], AOT.add)
            tt(t1[:], t1[:], lt[(2, 3)][:], AOT.add)
            ts(r[3][:], t1[:], -1.0, AOT.mult, 3.0, AOT.add)

            tt(t1[:], rem[0][:], rem[1][:], AOT.add)
            tt(t1[:], t1[:], rem[2][:], AOT.add)
            tt(t1[:], t1[:], rem[3][:], AOT.add)
            ts(t1[:], t1[:], 0.25, AOT.mult)
            for i in range(4):
                tt(r[i][:], r[i][:], t1[:], AOT.add)
            for i in range(4):
                ts(t2[:], r[i][:], 0.0, AOT.is_lt)
                ts(t3[:], r[i][:], 3.0, AOT.is_gt)
                stt(rem[i][:], t2[:], 4.0, AOT.mult, rem[i][:], AOT.add)
                stt(rem[i][:], t3[:], -4.0, AOT.mult, rem[i][:], AOT.add)
                stt(r[i][:], t2[:], 4.0, AOT.mult, r[i][:], AOT.add)
                stt(r[i][:], t3[:], -4.0, AOT.mult, r[i][:], AOT.add)

            delta = [TT(f"dl{i}") for i in range(4)]
            for i in range(4):
                tt(delta[i][:], e[i][:], rem[i][:], AOT.subtract)
                ts(delta[i][:], delta[i][:], 0.25, AOT.mult)

            sels = []
            for rv in range(4):
                acc = TT(f"sel{rv}")
                for i in range(4):
                    ts(t1[:], r[i][:], float(rv), AOT.is_equal)
                    tt(t1[:], t1[:], delta[i][:], AOT.mult)
                    if i == 0:
                        nc.vector.tensor_copy(acc[:], t1[:])
                    else:
                        tt(acc[:], acc[:], t1[:], AOT.add)
                sels.append(acc)
            mtile2 = hp.tile([128, 128], F32, tag="msk", name="msk_b4")
            nc.sync.dma_start(mtile2[:], msk[h * 16384 : (h + 1) * 16384].rearrange("(p f) -> p f", p=128))
            wsl = [RW[k][:, h * 128 : (h + 1) * 128] for k in range(4)]
            ts(t1[:], sels[0][:], -1.0, AOT.mult, 1.0, AOT.add)
            tt(wsl[0], sels[3][:], t1[:], AOT.add)
            tt(wsl[1], sels[2][:], sels[3][:], AOT.subtract)
            tt(wsl[2], sels[1][:], sels[2][:], AOT.subtract)
            tt(wsl[3], sels[0][:], sels[1][:], AOT.subtract)
            for k in range(4):
                tt(wsl[k], wsl[k], mtile2[:], AOT.mult)

            ges = {}
            for i in range(3):
                for th in (1, 2, 3):
                    g = TT(f"ge{i}{th}")
                    ts(g[:], r[i][:], float(th), AOT.is_ge)
                    ges[(i, th)] = g

            key = TT("key"); u = TT("u"); a = TT("a"); hsum = TT("hsum"); m10 = TT("m10")
            for k in range(4):
                for i in range(3):
                    if k == 0:
                        src = rem[i]
                    else:
                        stt(key[:], ges[(i, 4 - k)][:], -4.0, AOT.mult, rem[i][:], AOT.add)
                        ts(key[:], key[:], float(k), AOT.add)
                        src = key
                    Ah, Al = MULTS[i] // 1024, MULTS[i] % 1024
                    ts(u[:], src[:], float(Ah), AOT.mult)
                    mod_pow2(m10, u, 1024.0, t1, t4)
                    ts(a[:], src[:], float(Al), AOT.mult)
                    stt(a[:], m10[:], 1024.0, AOT.mult, a[:], AOT.add)
                    if i == 0:
                        nc.vector.tensor_copy(hsum[:], a[:])
                    else:
                        tt(hsum[:], hsum[:], a[:], AOT.add)
                slot = TT(f"slot{k}")
                mod_pow2(slot, hsum, float(CAP), t1, t4)

                # ---- bucket id / low 15 bits ----
                tc_id = h * DP1 + k
                bsl = RBk[k][:, h * 128 : (h + 1) * 128]
                ts(t1[:], slot[:], 1.0 / WIN, AOT.mult)
                f_round(t2[:], t1[:])
                tt(t4[:], t2[:], t1[:], AOT.is_gt)
                tt(bsl, t2[:], t4[:], AOT.subtract)          # b = floor(slot/32768)
                stt(RLOW[k][:, h * 128 : (h + 1) * 128], bsl, -float(WIN), AOT.mult, slot[:], AOT.add)

                # ---- per-bucket running counts (free-dim scan) ----
                Trows = hp.tile([128, 32], F32, tag="trows", name="trows_b4")
                ij = TT("ij"); sj = TT("sj"); tmp = TT("tmpsel")
                risl = RI[k][:, h * 128 : (h + 1) * 128]
                for j in range(NB):
                    ts(ij[:], bsl, float(j), AOT.is_equal)
                    nc.vector.tensor_tensor_scan(
                        out=sj[:], data0=ij[:], data1=ij[:], initial=0.0,
                        op0=AOT.add, op1=AOT.bypass,
                    )
                    tt(tmp[:], ij[:], sj[:], AOT.mult)
                    if j == 0:
                        nc.vector.tensor_copy(risl, tmp[:])
                    else:
                        tt(risl, risl, tmp[:], AOT.add)
                    nc.vector.tensor_copy(Trows[:, j : j + 1], sj[:, 127:128])

                pb = hpsum.tile([128, 32], F32, tag="pb", space="PSUM", name="pb_b4")
                nc.tensor.matmul(out=pb[:], lhsT=ltut[:], rhs=Trows[:], start=True, stop=True)
                nc.scalar.copy(PBres[:, tc_id * 32 : (tc_id + 1) * 32], pb[:])
                tb = hpsum.tile([128, 32], F32, tag="tb", space="PSUM", name="tb_b4")
                nc.tensor.matmul(out=tb[:], lhsT=onest[:], rhs=Trows[:], start=True, stop=True)
                nc.scalar.copy(TOTf[0:1, tc_id * 32 : (tc_id + 1) * 32], tb[0:1, :])

        s1ctx.close()

        # ---- Global scan over (tile, class); fold bases into PBres ----
        nc.vector.memset(GBf[0:1, 0:32], 0.0)
        for t in range(1, TC):
            tt(GBf[0:1, t * 32 : (t + 1) * 32],
               GBf[0:1, (t - 1) * 32 : t * 32],
               TOTf[0:1, (t - 1) * 32 : t * 32], AOT.add)
        ntf = midp.tile([1, 32], F32, tag="ntf", name="ntf")
        tt(ntf[:], GBf[0:1, (TC - 1) * 32 : TC * 32],
           TOTf[0:1, (TC - 1) * 32 : TC * 32], AOT.add)
        ts(ntf[:], ntf[:], float(CBCK), AOT.min)
        nc.vector.tensor_copy(NTOTi[:], ntf[:])
        nc.vector.memset(GBrhs[:], 0.0)
        gpsb = ctx.enter_context(tc.tile_pool(name="gps", bufs=2, space="PSUM"))
        for t in range(TC):
            nc.vector.tensor_copy(GBrhs[0:1, :], GBf[0:1, t * 32 : (t + 1) * 32])
            gbb = gpsb.tile([128, 32], F32, tag="gbb", space="PSUM", name="gbb_g4")
            nc.tensor.matmul(out=gbb[:], lhsT=e0[:].to_broadcast([128, 128]), rhs=GBrhs[:], start=True, stop=True)
            tt(PBres[:, t * 32 : (t + 1) * 32], PBres[:, t * 32 : (t + 1) * 32], gbb[:], AOT.add)

        # ---- Sweep 2: finish ranks, compute offsets, scatter to staging ----
        s2ctx = ExitStack()
        sp = s2ctx.enter_context(tc.tile_pool(name="s2", bufs=2))
        spp = s2ctx.enter_context(tc.tile_pool(name="s2ps", bufs=4, space="PSUM"))
        vp = s2ctx.enter_context(tc.tile_pool(name="s2v", bufs=2))
        vals_flat = vals.rearrange("n d -> (n d)")

        for _tr in range(t_reps):
         for h in range(NT):
            nblk = 128 // kblk
            vts = []
            for t0 in range(nblk):
                vt = vp.tile([128, kblk, 64], F32, tag=f"vt{t0}", name=f"vt{t0}_b4")
                nc.sync.dma_start(
                    vt[:],
                    vals_flat[h * 1048576 : (h + 1) * 1048576]
                    .rearrange("(p t c) -> p t c", p=128, t=128)[:, t0 * kblk : (t0 + 1) * kblk, :],
                )
                vts.append(vt)
            for k in range(4):
                tc_id = h * DP1 + k
                bsl = RBk[k][:, h * 128 : (h + 1) * 128]
                risl = RI[k][:, h * 128 : (h + 1) * 128]
                cmb = PBres[:, tc_id * 32 : (tc_id + 1) * 32]

                bsel = sp.tile([128, 128], F32, tag="bsel", name="bsel_b4")
                ij2 = sp.tile([128, 128], F32, tag="ij2", name="ij2_b4")
                tmp2 = sp.tile([128, 128], F32, tag="tmp2", name="tmp2_b4")
                for j in range(NB):
                    ts(ij2[:], bsl, float(j), AOT.is_equal)
                    tt(tmp2[:], ij2[:], cmb[:, j : j + 1].to_broadcast([128, 128]), AOT.mult)
                    if j == 0:
                        nc.vector.tensor_copy(bsel[:], tmp2[:])
                    else:
                        tt(bsel[:], bsel[:], tmp2[:], AOT.add)

                br = sp.tile([128, 128], F32, tag="br", name="br_b4")
                stt(br[:], risl, 1.0, AOT.subtract, bsel[:], AOT.add)
                ts(br[:], br[:], float(CBCK - 1), AOT.min)

                t1b = sp.tile([128, 128], F32, tag="t1b", name="t1b_b4")
                t4b = sp.tile([128, 128], F32, tag="t4b", name="t4b_b4")
                rm = sp.tile([128, 128], F32, tag="rm", name="rm_b4")
                ts(t1b[:], br[:], 1.0 / 128, AOT.mult)
                f_round(rm[:], t1b[:])
                tt(t4b[:], rm[:], t1b[:], AOT.is_gt)
                tt(rm[:], rm[:], t4b[:], AOT.subtract)
                stt(rm[:], rm[:], -128.0, AOT.mult, br[:], AOT.add)   # rm = br % 128
                av = sp.tile([128, 128], F32, tag="av", name="av_b4")
                tt(av[:], br[:], rm[:], AOT.subtract)
                ts(av[:], av[:], 1.0 / 128, AOT.mult)                 # br // 128
                spay = sp.tile([128, 128], F32, tag="spay", name="spay_b4")
                stt(spay[:], rm[:], float(CBP), AOT.mult, av[:], AOT.add)
                stt(spay[:], bsl, float(CBCK), AOT.mult, spay[:], AOT.add)

                ts(t1b[:], br[:], 1.0 / 16, AOT.mult)
                f_round(rm[:], t1b[:])
                tt(t4b[:], rm[:], t1b[:], AOT.is_gt)
                tt(rm[:], rm[:], t4b[:], AOT.subtract)
                stt(rm[:], rm[:], -16.0, AOT.mult, br[:], AOT.add)    # rm = br % 16
                tt(av[:], br[:], rm[:], AOT.subtract)
                ts(av[:], av[:], 1.0 / 16, AOT.mult)                  # br // 16
                sidx = sp.tile([128, 128], F32, tag="sidx", name="sidx_b4")
                stt(sidx[:], rm[:], float(CBI), AOT.mult, av[:], AOT.add)
                stt(sidx[:], bsl, float(CBCK), AOT.mult, sidx[:], AOT.add)

                oip = sp.tile([128, 128], I32, tag="oip", name="oip_b4")
                nc.vector.tensor_copy(oip[:], spay[:])
                oii = sp.tile([128, 128], I32, tag="oii", name="oii_b4")
                nc.vector.tensor_copy(oii[:], sidx[:])

                for t0 in range(nblk):
                    prod = sp.tile([128, kblk, 65], F32, tag=f"prod{t0 % 2}", name=f"prod{t0 % 2}_b4")
                    wsl = RW[k][:, h * 128 + t0 * kblk : h * 128 + (t0 + 1) * kblk]
                    tt(prod[:, :, 0:64], vts[t0][:], wsl.to_broadcast([128, kblk, 64]), AOT.mult)
                    nc.vector.tensor_copy(prod[:, :, 64:65], wsl.rearrange("p (a b) -> p a b", b=1))
                    nc.gpsimd.indirect_dma_start(
                        out=stg_pay[tc_id % pay_split][:],
                        out_offset=bass.IndirectOffsetOnAxis(ap=oip[:, t0 * kblk : (t0 + 1) * kblk], axis=0),
                        in_=prod[:],
                        in_offset=None,
                    )
                    nc.gpsimd.indirect_dma_start(
                        out=stg_idx[:],
                        out_offset=bass.IndirectOffsetOnAxis(ap=oii[:, t0 * kblk : (t0 + 1) * kblk], axis=0),
                        in_=RLOW[k][:, h * 128 + t0 * kblk : h * 128 + (t0 + 1) * kblk].rearrange("p (a b) -> p a b", b=1),
                        in_offset=None,
                    )

        s2ctx.close()
        mid.close()

        # ---- Pass 2: per bucket, load staged rows + one dma_scatter_add ----
        p2ctx = ExitStack()
        gp = p2ctx.enter_context(tc.tile_pool(name="p2", bufs=2))
        for _sr in range(s_reps):
         for b in range(NB):
            lps = []
            for s in range(pay_split):
                lp = gp.tile([128, CBP * 65], F32, tag=f"lp{s}", name=f"lp{s}_b4")
                nc.sync.dma_start(
                    lp[:],
                    stg_pay[s].rearrange("r c -> (r c)")[b * CBCK * 65 : (b + 1) * CBCK * 65]
                    .rearrange("(p f) -> p f", p=128),
                )
                lps.append(lp)
            if pay_split > 1:
                for s in range(1, pay_split):
                    tt(lps[0][:], lps[0][:], lps[s][:], AOT.add)
            ixf = gp.tile([128, CBI], F32, tag="ixf", name="ixf_b4")
            nc.sync.dma_start(
                ixf[0:16, :],
                stg_idx.rearrange("r c -> (r c)")[b * CBCK : (b + 1) * CBCK]
                .rearrange("(p f) -> p f", p=16),
            )
            ixi = gp.tile([128, CBI], mybir.dt.int16, tag="ixi", name="ixi_b4")
            nc.vector.memset(ixi[:], -1.0)
            nc.vector.tensor_copy(ixi[0:16, :], ixf[0:16, :])
            nreg = nc.gpsimd.value_load(NTOTi[0:1, b : b + 1], min_val=0, max_val=CBCK)
            nc.gpsimd.dma_scatter_add(
                out_ap=scratch[b * WIN : (b + 1) * WIN, 0:65],
                in_ap=lps[0][:].rearrange("p (a b) -> p a b", b=65),
                idxs_ap=ixi[:],
                num_idxs=CBCK,
                num_idxs_reg=nreg,
                elem_size=65,
                elem_step=128,
            )
        p2ctx.close()

        # ---- Compact: scratch [CAP,128] -> out [CAP,65] ----
        cctx = ExitStack()
        cp = cctx.enter_context(tc.tile_pool(name="cp4", bufs=4))
        sflat = scratch.rearrange("r c -> (r c)")
        oflat = out.rearrange("r c -> (r c)")
        RPI = 4096                      # rows per compact iteration
        for _cr in range(c_reps):
         for i in range(CAP // RPI):
            rt = cp.tile([128, RPI * 128 // 128], F32, tag="rt", name="rt_b4")
            nc.sync.dma_start(
                rt[:],
                sflat[i * RPI * 128 : (i + 1) * RPI * 128].rearrange("(p f) -> p f", p=128),
            )
            ctt = cp.tile([128, RPI * 65 // 128], F32, tag="ct", name="ct_b4")
            nc.vector.tensor_copy(
                ctt[:].rearrange("p (a b) -> p a b", b=65),
                rt[:].rearrange("p (a b) -> p a b", b=128)[:, :, 0:65],
            )
            nc.sync.dma_start(
                oflat[i * RPI * 65 : (i + 1) * RPI * 65].rearrange("(p f) -> p f", p=128),
                ctt[:],
            )
        cctx.close()


def make_core_inputs(pos_shard, val_shard, NP):
    """Pad a core's shard to NP points and build the input map."""
    n = pos_shard.shape[0]
    assert n <= NP
    pos = np.zeros((NP, 3), np.float32)
    pos[:n] = pos_shard
    valp = np.zeros((NP, 64), np.float32)
    valp[:n] = val_shard
    m = np.zeros((NP,), np.float32)
    m[:n] = 1.0
    return {
        "positions": pos.reshape(-1),
        "values": valp,
        "mask": m,
        "ident": np.eye(128, dtype=np.float32),
        "ltm": np.tril(np.ones((128, 128), np.float32), -1),
        "ltu": np.triu(np.ones((128, 128), np.float32), 1),
        "capp": (CAP + np.arange(128, dtype=np.float32)).reshape(128, 1),
    }


from concourse.bass_utils import run_bass_kernel_spmd

N_CORES = 8
_CACHE = {}

# Active kernel configuration ("builder" selects build()/build2()).
CONFIG = dict(builder="v2", merge_mode="host", zero_mode="donated")


def build_cfg(nc, NP, cfg):
    cfg = dict(cfg)
    b = cfg.pop("builder", "v1")
    if b == "v4":
        build4(nc, NP, **cfg)
    elif b == "v2":
        build2(nc, NP, **cfg)
    else:
        build(nc, NP, **cfg)


def _get_program(NP):
    key = (NP, tuple(sorted(CONFIG.items())))
    if key not in _CACHE:
        nc = bacc.Bacc("TRN2", target_bir_lowering=False, debug=False, num_devices=N_CORES)
        build_cfg(nc, NP, CONFIG)
        nc.compile()
        _CACHE[key] = nc
    return _CACHE[key]


def kernel(positions, values, hash_capacity):
    positions = np.ascontiguousarray(np.asarray(positions, dtype=np.float32))
    values = np.ascontiguousarray(np.asarray(values, dtype=np.float32))
    assert int(hash_capacity) == CAP, f"kernel compiled for capacity {CAP}"
    n = positions.shape[0]
    nsh = (n + N_CORES - 1) // N_CORES
    NP = ((nsh + 16383) // 16384) * 16384

    nc = _get_program(NP)

    in_maps = []
    for c in range(N_CORES):
        lo, hi = c * nsh, min((c + 1) * nsh, n)
        in_maps.append(
            make_core_inputs(positions[lo:hi], values[lo:hi], NP)
        )

    res = run_bass_kernel_spmd(nc, in_maps, core_ids=list(range(N_CORES)))

    acc = np.zeros((CAP, 65), np.float64)
    for c in range(N_CORES):
        r = res.results[c]
        if "out" in r and r["out"].shape[0] >= CAP and r["out"].shape[0] < 2 * CAP:
            acc += r["out"][:CAP].astype(np.float64)
        else:  # concat table [4*CAP(+..), 65]: fold classes on host
            o = r["out"]
            for k in range(4):
                acc += o[k * CAP : (k + 1) * CAP].astype(np.float64)
        for nm in r:
            if nm.startswith("tab"):
                acc += r[nm][:CAP].astype(np.float64)
    return np.ascontiguousarray(acc.astype(np.float32))

